# revision 18
# baseline (speedup 1.0000x reference)
"""DGCNN-ReID forward pass on 8 Trainium2 NeuronCores (Bass/Tile).

Data-parallel over batch (16 samples -> 2 per core). Per EdgeConv layer:
the kNN affinity matrix is built on the TensorEngine, exact top-20
neighbor indices are extracted with DVE max8/max_index/match_replace
rounds (stable-argsort semantics, tie-safe), and neighbor features are
fetched with GPSIMD indirect_copy gathers from a feature-major u table,
followed by a max-reduce over the 20 neighbors. Training-mode BatchNorm
statistics: layers 1-2 accumulate E[y], E[y^2] directly from the gathered
edge values (avoids catastrophic cancellation of the sum-decomposition at
small feature scales); layers 3-4 use one cumulative top-k mask matmul
per sample (bf16) for the neighbor sums. Stats are AllReduced across the
8 cores.
"""
import sys
sys.path.insert(0, '/opt/trn_rl_repo')
import numpy as np
from contextlib import ExitStack

import concourse.bass as bass
import concourse.tile as tile
from concourse import bacc, mybir
from concourse import bass_utils

dt = mybir.dt
F32 = dt.float32
F32R = dt.float32r
BF16 = dt.bfloat16
U16 = dt.uint16
AF = mybir.ActivationFunctionType
ALU = mybir.AluOpType
AX = mybir.AxisListType

N_CORES = 8
B, BL, N, K = 16, 2, 1024, 20
NQ = N // 128
EPS = 1e-5
SLOPE = 0.2
EMB = 1024
CLS = 751
LAYERS = [(3, 64), (64, 64), (64, 128), (128, 256)]
DIRECT = [True, True, False, False]
CNT_EDGE = float(B * N * K)
CNT_Y = float(B * N)
CNT_H = float(B)
NEG = -1e30


def R(ap):
    return ap.bitcast(F32R)


def kb_blocks(C):
    out = []
    c = 0
    while c < C:
        out.append((c, min(128, C - c)))
        c += 128
    return out


def build():
    nc = bacc.Bacc("TRN2", target_bir_lowering=False, debug=False,
                   enable_asserts=True, num_devices=N_CORES)

    ptsT = nc.dram_tensor("ptsT", [BL, 3, N], F32, kind="ExternalInput").ap()
    wn_d, wd_d, gm_d, bt_d = [], [], [], []
    for li, (C, O) in enumerate(LAYERS):
        wn_d.append(nc.dram_tensor(f"wn{li}", [C, O], F32, kind="ExternalInput").ap())
        wd_d.append(nc.dram_tensor(f"wd{li}", [C, O], F32, kind="ExternalInput").ap())
        gm_d.append(nc.dram_tensor(f"g{li}", [O], F32, kind="ExternalInput").ap())
        bt_d.append(nc.dram_tensor(f"b{li}", [O], F32, kind="ExternalInput").ap())
    w5T_d = nc.dram_tensor("w5T", [512, EMB], F32, kind="ExternalInput").ap()
    g5_d = nc.dram_tensor("g5", [EMB], F32, kind="ExternalInput").ap()
    b5_d = nc.dram_tensor("b5", [EMB], F32, kind="ExternalInput").ap()
    fw1T_d = nc.dram_tensor("fw1T", [2 * EMB + 1, 256], F32, kind="ExternalInput").ap()
    fg1_d = nc.dram_tensor("fg1", [256], F32, kind="ExternalInput").ap()
    fbb1_d = nc.dram_tensor("fbb1", [256], F32, kind="ExternalInput").ap()
    fw2T_d = nc.dram_tensor("fw2T", [257, 256], F32, kind="ExternalInput").ap()
    fg2_d = nc.dram_tensor("fg2", [256], F32, kind="ExternalInput").ap()
    fbb2_d = nc.dram_tensor("fbb2", [256], F32, kind="ExternalInput").ap()
    fw3T_d = nc.dram_tensor("fw3T", [257, CLS], F32, kind="ExternalInput").ap()
    out_d = nc.dram_tensor("out", [BL, CLS], F32, kind="ExternalOutput").ap()

    with tile.TileContext(nc) as tc, ExitStack() as ctx:
        sb = ctx.enter_context(tc.tile_pool(name="sb", bufs=1))
        wk = ctx.enter_context(tc.tile_pool(name="wk", bufs=1))
        ps = ctx.enter_context(tc.tile_pool(name="ps", bufs=8, space="PSUM"))
        dr = ctx.enter_context(tc.tile_pool(name="dr", bufs=1, space="DRAM"))

        ones_row = sb.tile([1, N], F32)
        nc.vector.memset(ones_row[:], 1.0)
        ones2 = sb.tile([1, 2], F32)
        nc.vector.memset(ones2[:], 1.0)
        iot = sb.tile([128, 128], F32)
        nc.gpsimd.iota(iot[:], pattern=[[1, 128]], base=0, channel_multiplier=-1,
                       allow_small_or_imprecise_dtypes=True)
        identb = sb.tile([128, 128], BF16)
        nc.vector.tensor_scalar(identb[:], iot[:], 0.0, None, ALU.is_equal)

        # ---- load weights ----
        wn_sb, wd_sb = [], []
        for li, (C, O) in enumerate(LAYERS):
            t1 = sb.tile([C, O], F32, name=f"wn{li}")
            nc.sync.dma_start(t1[:], wn_d[li][:, :])
            wn_sb.append(t1)
            t2 = sb.tile([C, O], F32, name=f"wd{li}")
            nc.sync.dma_start(t2[:], wd_d[li][:, :])
            wd_sb.append(t2)
        W5BLK = [(0, 64), (64, 64), (128, 128), (256, 128), (384, 128)]
        fb1row = sb.tile([1, 256], F32)
        nc.sync.dma_start(fb1row[:], fw1T_d[2 * EMB:2 * EMB + 1, :])
        fw2_sb = []
        for i, (c0, cb) in enumerate(kb_blocks(256)):
            t = sb.tile([cb, 256], F32, name=f"fw2_{i}")
            nc.sync.dma_start(t[:], fw2T_d[c0:c0 + cb, :])
            fw2_sb.append(t)
        fb2row = sb.tile([1, 256], F32)
        nc.sync.dma_start(fb2row[:], fw2T_d[256:257, :])
        fw3_sb = []
        for i, (c0, cb) in enumerate(kb_blocks(256)):
            t = sb.tile([cb, CLS], F32, name=f"fw3_{i}")
            nc.sync.dma_start(t[:], fw3T_d[c0:c0 + cb, :])
            fw3_sb.append(t)
        fb3row = sb.tile([1, CLS], F32)
        nc.sync.dma_start(fb3row[:], fw3T_d[256:257, :])

        # persistent x_l.T feature tiles (layer 0 input = pts)
        xs = [[sb.tile([LAYERS[li][0], N], F32, name=f"x{li}_{s}")
               for s in range(BL)] for li in range(4)]
        x4 = [[sb.tile([128, N], F32, name=f"x4_{s}_{i}") for i in range(2)]
              for s in range(BL)]
        for s in range(BL):
            nc.sync.dma_start(xs[0][s][:], ptsT[s])

        def xnext_tiles(li, s):
            # output z.T destination tiles for layer li: list of (tile, ob)
            if li + 1 < 4:
                return [(xs[li + 1][s], LAYERS[li][1])]
            return [(x4[s][0], 128), (x4[s][1], 128)]

        # ================= edge layers =================
        for li, (C, O) in enumerate(LAYERS):
            Mt = (O + 127) // 128
            direct = DIRECT[li]
            if direct:
                syp = wk.tile([128, BL * 4], F32, tag="syp")
                sy2p = wk.tile([128, BL * 4], F32, tag="sy2p")
            else:
                sup = wk.tile([128, Mt, BL * NQ], F32, tag="syp")
                sqp = wk.tile([128, Mt, BL * NQ], F32, tag="sy2p")
                crp = wk.tile([128, Mt, BL * NQ], F32, tag="crp")
                scp = wk.tile([128, Mt, BL * 2], F32, tag="scp")
                scqp = wk.tile([128, Mt, BL * 2], F32, tag="scqp")

            for s in range(BL):
                xb = xs[li][s]
                xsq = wk.tile([C, N], F32, tag="xsq", bufs=2)
                nc.scalar.activation(xsq[:], xb[:], AF.Square)
                onesc = wk.tile([C, 1], F32, tag="onesc", bufs=2)
                nc.vector.memset(onesc[:], 1.0)
                xxneg = wk.tile([1, N], F32, tag="xxneg", bufs=2)
                for ch in range(2):
                    cs = slice(512 * ch, 512 * (ch + 1))
                    pxx = ps.tile([1, 512], F32, name=f"pxx{li}{s}{ch}", tag="ps")
                    nc.tensor.matmul(pxx[:], onesc[:], xsq[:, cs], start=True,
                                     stop=True)
                    nc.scalar.activation(xxneg[:, cs], pxx[:], AF.Identity,
                                         scale=-0.5)

                # u.T (f32 gather table)
                uTs = []
                for m in range(Mt):
                    ob = min(128, O - 128 * m)
                    ut = wk.tile([128, N], F32, tag="uT0", bufs=2)
                    for ch in range(2):
                        cs = slice(512 * ch, 512 * (ch + 1))
                        pu = ps.tile([ob, 512], F32, name=f"pu{li}{s}{m}{ch}",
                                     tag="ps")
                        nc.tensor.matmul(pu[:], wn_sb[li][:, 128 * m:128 * m + ob],
                                         xb[:, cs], start=True, stop=True)
                        nc.scalar.activation(ut[0:ob, cs], pu[:], AF.Copy)
                    uTs.append(ut)
                if O == 64:
                    nc.sync.dma_start(uTs[0][64:128, :], uTs[0][0:64, :])

                # c.T = ((wc-wn).T x).T
                cT = wk.tile([128, Mt, N], F32, tag="cT")
                for m in range(Mt):
                    ob = min(128, O - 128 * m)
                    for ch in range(2):
                        cs = slice(512 * ch, 512 * (ch + 1))
                        pc = ps.tile([ob, 512], F32, name=f"pc{li}{s}{m}{ch}",
                                     tag="ps")
                        nc.tensor.matmul(pc[:], wd_sb[li][:, 128 * m:128 * m + ob],
                                         xb[:, cs], start=True, stop=True)
                        if direct:
                            nc.scalar.activation(cT[0:ob, m, cs], pc[:], AF.Copy)
                        else:
                            nc.scalar.activation(cT[0:ob, m, cs], pc[:],
                                                 AF.Identity,
                                                 accum_out=scp[0:ob, m,
                                                              2 * s + ch:2 * s + ch + 1])
                            tr = wk.tile([128, 512], F32, tag="trash", bufs=2)
                            nc.scalar.activation(tr[0:ob, :], cT[0:ob, m, cs],
                                                 AF.Square,
                                                 accum_out=scqp[0:ob, m,
                                                               2 * s + ch:2 * s + ch + 1])
                cd = None
                if direct:
                    # rows 0-63: c.T; rows 64-127: c.T shifted left by 128 cols
                    cd = wk.tile([128, N], F32, tag="cdup")
                    nc.sync.dma_start(cd[0:64, :], cT[0:64, 0, :])
                    nc.sync.dma_start(cd[64:128, 0:N - 128], cT[0:64, 0, 128:N])

                # point-major u and u^2 (bf16) for mask-stat matmuls
                if not direct:
                    upm = wk.tile([128, NQ, O], BF16, tag="upm")
                    usq = wk.tile([128, NQ, O], BF16, tag="usq")
                    for ib in range(NQ):
                        pp = ps.tile([128, O], F32, name=f"pp{li}{s}{ib}", tag="ps")
                        nc.tensor.matmul(pp[:], xb[:, 128 * ib:128 * (ib + 1)],
                                         wn_sb[li][:, :], start=True, stop=True)
                        nc.scalar.activation(upm[:, ib, :], pp[:], AF.Copy)
                        nc.scalar.activation(usq[:, ib, :], upm[:, ib, :], AF.Square)

                # ---- affinity + top-20 indices per dest block ----
                # top-20 indices, contiguous 20 per block; layers 1-2 use
                # parity-major column order so the DRAM-bounce DMAs merge.
                i24 = wk.tile([128, NQ, 20], U16, tag="i24", bufs=2)
                i24f = i24[:].rearrange("p a b -> p (a b)")
                for q in range(NQ):
                    qs = slice(128 * q, 128 * (q + 1))
                    c0 = ((q % 2) * 80 + (q // 2) * 20) if O == 64 else 20 * q
                    sq_ = wk.tile([128, N], F32, tag="sq", bufs=3)
                    for ch in range(2):
                        cs = slice(512 * ch, 512 * (ch + 1))
                        pss = ps.tile([128, 512], F32, name=f"pss{li}{s}{q}{ch}",
                                      tag="ps")
                        nc.tensor.matmul(pss[:], xb[:, qs], xb[:, cs],
                                         start=True, stop=False)
                        nc.tensor.matmul(pss[:], ones_row[:, qs], xxneg[:, cs],
                                         start=False, stop=True)
                        nc.scalar.activation(sq_[:, cs], pss[:], AF.Copy)
                    v24 = wk.tile([128, 24], F32, tag="v24", bufs=2)
                    nc.vector.max(v24[:, 0:8], sq_[:])
                    nc.vector.max_index(i24f[:, c0:c0 + 8], v24[:, 0:8], sq_[:])
                    sq2 = wk.tile([128, N], F32, tag="sq", bufs=3)
                    nc.vector.match_replace(sq2[:], v24[:, 0:8], sq_[:], NEG)
                    nc.vector.max(v24[:, 8:16], sq2[:])
                    nc.vector.max_index(i24f[:, c0 + 8:c0 + 16], v24[:, 8:16], sq2[:])
                    sq3 = wk.tile([128, N], F32, tag="sq", bufs=3)
                    nc.vector.match_replace(sq3[:], v24[:, 8:16], sq2[:], NEG)
                    nc.vector.max(v24[:, 16:24], sq3[:])
                    i8 = wk.tile([128, 8], U16, tag="i8", bufs=2)
                    nc.vector.max_index(i8[:], v24[:, 16:24], sq3[:])
                    nc.vector.tensor_copy(i24f[:, c0 + 16:c0 + 20], i8[:, 0:4])
                    if not direct:
                        mq = wk.tile([128, N], BF16, tag="mskq")
                        nc.vector.tensor_scalar(mq[:], sq_[:], v24[:, 19:20],
                                                None, ALU.is_ge)
                        mskq_t = wk.tile([128, NQ, 128], BF16, tag="mskT", bufs=2)
                        for jb in range(NQ):
                            pt = ps.tile([128, 128], BF16,
                                         name=f"pt{li}{s}{q}{jb}", tag="ps")
                            nc.tensor.transpose(pt[:], mq[:, 128 * jb:128 * (jb + 1)],
                                                identb[:])
                            nc.scalar.activation(mskq_t[:, jb, :], pt[:], AF.Copy)
                        for m in range(Mt):
                            ob = min(128, O - 128 * m)
                            ms = slice(128 * m, 128 * m + ob)
                            pS = ps.tile([128, 128], F32,
                                         name=f"pS{li}{s}{q}{m}", tag="ps")
                            pQ = ps.tile([128, 128], F32,
                                         name=f"pQ{li}{s}{q}{m}", tag="ps")
                            for jb in range(NQ):
                                nc.tensor.matmul(pS[0:ob, :], upm[:, jb, ms],
                                                 mskq_t[:, jb, :],
                                                 start=(jb == 0), stop=(jb == NQ - 1))
                                nc.tensor.matmul(pQ[0:ob, :], usq[:, jb, ms],
                                                 mskq_t[:, jb, :],
                                                 start=(jb == 0), stop=(jb == NQ - 1))
                            ci = s * NQ + q
                            tr = wk.tile([128, 512], F32, tag="trash", bufs=2)
                            nc.scalar.activation(tr[0:ob, 0:128], pS[0:ob, :],
                                                 AF.Identity,
                                                 accum_out=sup[0:ob, m, ci:ci + 1])
                            tr2 = wk.tile([128, 512], F32, tag="trash", bufs=2)
                            nc.scalar.activation(tr2[0:ob, 0:128], pQ[0:ob, :],
                                                 AF.Identity,
                                                 accum_out=sqp[0:ob, m, ci:ci + 1])
                            trd = wk.tile([128, 512], F32, tag="trash", bufs=2)
                            nc.vector.scalar_tensor_tensor(
                                trd[0:ob, 0:128], pS[0:ob, :], 1.0,
                                cT[0:ob, m, qs], ALU.mult, ALU.mult,
                                accum_out=crp[0:ob, m, ci:ci + 1])

                # ---- index wrap for gpsimd gathers (via DRAM bounce) ----
                # k-major gather order: list element t = 128*k + i so that the
                # DRAM->SBUF wrap DMA strides merge. G comes out as [ch, k, i].
                if O == 64:
                    # pair-stacked: groups 0-3 <- blocks 0,2,4,6 (parity 0),
                    # groups 4-7 <- blocks 1,3,5,7 (parity 1)
                    idxd = dr.tile([2, 4, 20, 128], U16, name=f"idxd{li}{s}")
                    for par in range(2):
                        s1ap = bass.AP(i24.tensor, i24f[:, 80 * par:80 * par + 80].offset,
                                       [i24[:].ap[0], [20, 4], [1, 20]])
                        d1ap = bass.AP(idxd.tensor, idxd[par].offset,
                                       [[1, 128], [2560, 4], [128, 20]])
                        nc.sync.dma_start(d1ap, s1ap)
                    idxw = wk.tile([128, 4, 160], U16, tag="idxw")
                    for g in range(8):
                        srcg = bass.AP(idxd.tensor, idxd[g // 4].offset,
                                       [[1, 16], [16, 640]])
                        dslc = idxw[16 * g:16 * (g + 1), :, :]
                        dstg = bass.AP(idxw.tensor, dslc.offset,
                                       [dslc.ap[0], [1, 640]])
                        nc.sync.dma_start(dstg, srcg)
                else:
                    idxd = dr.tile([NQ, 20, 128], U16, name=f"idxd{li}{s}")
                    d1ap = bass.AP(idxd.tensor, idxd[:].offset,
                                   [[1, 128], [2560, NQ], [128, 20]])
                    nc.sync.dma_start(d1ap, i24[:, :, :])
                    idxw = wk.tile([128, NQ, 160], U16, tag="idxw")
                    for g in range(8):
                        srcg = bass.AP(idxd.tensor, idxd[:].offset,
                                       [[1, 16], [16, 1280]])
                        dslc = idxw[16 * g:16 * (g + 1), :, :]
                        dstg = bass.AP(idxw.tensor, dslc.offset,
                                       [dslc.ap[0], [1, 1280]])
                        nc.sync.dma_start(dstg, srcg)

                def pool_kmax(G, ob):
                    # max over k: G is [128, 20(k), 128(i)]; a transposed AP
                    # view puts k innermost so one strided reduce does it.
                    Gv = G[:].rearrange("p (k i) -> p i k", i=128)
                    zt = wk.tile([128, 128], F32, tag="zt", bufs=2)
                    nc.vector.tensor_reduce(zt[0:ob, :], Gv[0:ob, :, :], AX.X,
                                            ALU.max)
                    return zt

                if direct:
                    for t in range(4):
                        G = wk.tile([128, 2560], F32, tag="G", bufs=2)
                        for a, b in [(0, 64), (64, 128), (128, 160)]:
                            nc.gpsimd.indirect_copy(
                                G[:, 16 * a:16 * b].rearrange("p (i o) -> p i o", o=1),
                                uTs[0][:], idxw[:, t, a:b], True)
                        cds = cd[:, 256 * t:256 * t + 128]
                        cb = bass.AP(cd.tensor, cds.offset,
                                     [cds.ap[0], [0, 20], [1, 128]])
                        tG = wk.tile([128, 2560], F32, tag="tG")
                        nc.vector.scalar_tensor_tensor(
                            tG[:].rearrange("p (k i) -> p k i", i=128),
                            G[:].rearrange("p (k i) -> p k i", i=128),
                            0.0, cb, ALU.add, ALU.add,
                            accum_out=syp[:, s * 4 + t:s * 4 + t + 1])
                        nc.scalar.activation(tG[:], tG[:], AF.Square,
                                             accum_out=sy2p[:, s * 4 + t:s * 4 + t + 1])
                        zt = pool_kmax(G, 128)
                        dst, _ = xnext_tiles(li, s)[0]
                        nc.vector.tensor_add(dst[0:64, 256 * t:256 * t + 128],
                                             zt[0:64, :], cd[0:64, 256 * t:256 * t + 128])
                        zhi = wk.tile([128, 128], F32, tag="zt", bufs=2)
                        nc.vector.tensor_add(zhi[64:128, :], zt[64:128, :],
                                             cd[64:128, 256 * t:256 * t + 128])
                        nc.sync.dma_start(dst[0:64, 256 * t + 128:256 * t + 256],
                                          zhi[64:128, :])
                else:
                    for q in range(NQ):
                        for m in range(Mt):
                            ob = min(128, O - 128 * m)
                            G = wk.tile([128, 2560], F32, tag="G", bufs=2)
                            for a, b in [(0, 64), (64, 128), (128, 160)]:
                                nc.gpsimd.indirect_copy(
                                    G[:, 16 * a:16 * b].rearrange("p (i o) -> p i o", o=1),
                                    uTs[m][:], idxw[:, q, a:b], True)
                            zt = pool_kmax(G, ob)
                            dst, _ = xnext_tiles(li, s)[m]
                            nc.vector.tensor_add(dst[0:ob, 128 * q:128 * (q + 1)],
                                                 zt[0:ob, :],
                                                 cT[0:ob, m, 128 * q:128 * (q + 1)])

            # ---- finalize stats ----
            stat = wk.tile([128, Mt, 2], F32, tag="stat")
            nc.vector.memset(stat[:].rearrange("p a b -> p (a b)"), 0.0)
            if direct:
                fold1 = wk.tile([64, BL * 4], F32, tag="fold1")
                nc.sync.dma_start(fold1[:], syp[64:128, :])
                fold2 = wk.tile([64, BL * 4], F32, tag="fold2")
                nc.sync.dma_start(fold2[:], sy2p[64:128, :])
                nc.vector.tensor_add(syp[0:64, :], syp[0:64, :], fold1[:])
                nc.vector.tensor_add(sy2p[0:64, :], sy2p[0:64, :], fold2[:])
                nc.vector.tensor_reduce(stat[0:64, 0, 0:1], syp[0:64, :], AX.X,
                                        ALU.add)
                nc.vector.tensor_reduce(stat[0:64, 0, 1:2], sy2p[0:64, :], AX.X,
                                        ALU.add)
            else:
                red = wk.tile([128, Mt, 5], F32, tag="red")
                for m in range(Mt):
                    ob = min(128, O - 128 * m)
                    nc.vector.tensor_reduce(red[0:ob, m, 0:1], sup[0:ob, m, :],
                                            AX.X, ALU.add)
                    nc.vector.tensor_reduce(red[0:ob, m, 1:2], sqp[0:ob, m, :],
                                            AX.X, ALU.add)
                    nc.vector.tensor_reduce(red[0:ob, m, 2:3], crp[0:ob, m, :],
                                            AX.X, ALU.add)
                    nc.vector.tensor_reduce(red[0:ob, m, 3:4], scp[0:ob, m, :],
                                            AX.X, ALU.add)
                    nc.vector.tensor_reduce(red[0:ob, m, 4:5], scqp[0:ob, m, :],
                                            AX.X, ALU.add)
                    # Sy = Su + K*Sc
                    nc.vector.tensor_scalar(stat[0:ob, m, 0:1], red[0:ob, m, 3:4],
                                            float(K), None, ALU.mult)
                    nc.vector.tensor_add(stat[0:ob, m, 0:1], stat[0:ob, m, 0:1],
                                         red[0:ob, m, 0:1])
                    # Sy2 = Sq + 2*cross + K*Scq
                    nc.vector.tensor_scalar(stat[0:ob, m, 1:2], red[0:ob, m, 2:3],
                                            2.0, None, ALU.mult)
                    nc.vector.tensor_add(stat[0:ob, m, 1:2], stat[0:ob, m, 1:2],
                                         red[0:ob, m, 1:2])
                    tk = wk.tile([128, 1], F32, tag="tk", bufs=2)
                    nc.vector.tensor_scalar(tk[0:ob, :], red[0:ob, m, 4:5],
                                            float(K), None, ALU.mult)
                    nc.vector.tensor_add(stat[0:ob, m, 1:2], stat[0:ob, m, 1:2],
                                         tk[0:ob, :])

            # ---- AllReduce + BN coefs + apply ----
            bin_ = dr.tile([128, Mt * 2], F32, name=f"bi{li}")
            bout = dr.tile([128, Mt * 2], F32, name=f"bo{li}")
            nc.sync.dma_start(bin_[:], stat[:].rearrange("p a b -> p (a b)"))
            nc.gpsimd.collective_compute("AllReduce", ALU.add,
                                         replica_groups=[list(range(N_CORES))],
                                         ins=[bin_.opt()], outs=[bout.opt()])
            statg = wk.tile([128, Mt, 2], F32, tag="statg")
            nc.sync.dma_start(statg[:].rearrange("p a b -> p (a b)"), bout[:])
            gamv = wk.tile([128, Mt], F32, tag="gamv")
            betv = wk.tile([128, Mt], F32, tag="betv")
            for m in range(Mt):
                ob = min(128, O - 128 * m)
                nc.sync.dma_start(gamv[0:ob, m:m + 1],
                                  gm_d[li][128 * m:128 * m + ob].rearrange("(p a) -> p a", a=1))
                nc.sync.dma_start(betv[0:ob, m:m + 1],
                                  bt_d[li][128 * m:128 * m + ob].rearrange("(p a) -> p a", a=1))
            av = wk.tile([128, Mt], F32, tag="av")
            cv = wk.tile([128, Mt], F32, tag="cv")
            nav = wk.tile([128, Mt], F32, tag="nav")
            ncv = wk.tile([128, Mt], F32, tag="ncv")
            tv = wk.tile([128, Mt, 4], F32, tag="tv")
            for m in range(Mt):
                ob = min(128, O - 128 * m)
                nc.vector.tensor_scalar(tv[0:ob, m, 0:1], statg[0:ob, m, 0:1],
                                        1.0 / CNT_EDGE, None, ALU.mult)
                nc.vector.tensor_scalar(tv[0:ob, m, 1:2], statg[0:ob, m, 1:2],
                                        1.0 / CNT_EDGE, None, ALU.mult)
                nc.vector.tensor_mul(tv[0:ob, m, 2:3], tv[0:ob, m, 0:1],
                                     tv[0:ob, m, 0:1])
                nc.vector.tensor_sub(tv[0:ob, m, 1:2], tv[0:ob, m, 1:2],
                                     tv[0:ob, m, 2:3])
                nc.vector.tensor_scalar(tv[0:ob, m, 1:2], tv[0:ob, m, 1:2], EPS,
                                        None, ALU.add)
                nc.vector.reciprocal(tv[0:ob, m, 2:3], tv[0:ob, m, 1:2])
                nc.scalar.activation(tv[0:ob, m, 3:4], tv[0:ob, m, 2:3], AF.Sqrt)
                nc.vector.tensor_mul(av[0:ob, m:m + 1], tv[0:ob, m, 3:4],
                                     gamv[0:ob, m:m + 1])
                nc.vector.tensor_mul(tv[0:ob, m, 2:3], av[0:ob, m:m + 1],
                                     tv[0:ob, m, 0:1])
                nc.vector.tensor_sub(cv[0:ob, m:m + 1], betv[0:ob, m:m + 1],
                                     tv[0:ob, m, 2:3])
                nc.vector.tensor_scalar(nav[0:ob, m:m + 1], av[0:ob, m:m + 1], -1.0,
                                        None, ALU.mult)
                nc.vector.tensor_scalar(ncv[0:ob, m:m + 1], cv[0:ob, m:m + 1], -1.0,
                                        None, ALU.mult)
            for s in range(BL):
                for m in range(Mt):
                    ob = min(128, O - 128 * m)
                    dst, _ = xnext_tiles(li, s)[m]
                    p_s = wk.tile([128, N], F32, tag="sq", bufs=3)
                    q_s = wk.tile([128, N], F32, tag="sq", bufs=3)
                    nc.scalar.activation(p_s[0:ob, :], dst[0:ob, :], AF.Relu,
                                         bias=cv[0:ob, m:m + 1],
                                         scale=av[0:ob, m:m + 1])
                    nc.scalar.activation(q_s[0:ob, :], dst[0:ob, :], AF.Relu,
                                         bias=ncv[0:ob, m:m + 1],
                                         scale=nav[0:ob, m:m + 1])
                    nc.vector.tensor_scalar(q_s[0:ob, :], q_s[0:ob, :], SLOPE,
                                            None, ALU.mult)
                    nc.vector.tensor_sub(dst[0:ob, :], p_s[0:ob, :], q_s[0:ob, :])

        # ================= conv5 + pooling =================
        w5_t = wk.tile([128, 5, EMB], F32R, tag="bigB", name="w5_t")
        for i, (c0, cb) in enumerate(W5BLK):
            w5_s = wk.tile([128, EMB], F32, tag="xsq", bufs=2)
            nc.sync.dma_start(w5_s[0:cb, :], w5T_d[c0:c0 + cb, :])
            nc.scalar.activation(w5_t[0:cb, i, :], w5_s[0:cb, :], AF.Copy)

        def xc_blocks(s):
            return [xs[1][s], xs[2][s], xs[3][s], x4[s][0], x4[s][1]]

        XCTAGS = [("cdup", 1), ("xsq", 2), ("uT0", 2), ("G", 2), ("uT0", 2)]

        def xc_rounded(s):
            out = []
            for i, t in enumerate(xc_blocks(s)):
                cb = t.shape[0]
                tg, bf = XCTAGS[i]
                rt = wk.tile([cb, N], F32R, name=f"xcr{s}_{i}", tag=tg, bufs=bf)
                nc.scalar.activation(rt[:], t[:], AF.Copy)
                out.append(rt)
            return out

        s1 = wk.tile([128, 8, 4], F32, tag="s1c")
        s2 = wk.tile([128, 8, 4], F32, tag="s2c")
        for s in range(BL):
            xcb = xc_rounded(s)
            for m in range(8):
                ms = slice(128 * m, 128 * (m + 1))
                for ch in range(2):
                    cs = slice(512 * ch, 512 * (ch + 1))
                    py = ps.tile([128, 512], F32, name=f"pw{s}{m}{ch}", tag="ps")
                    for i in range(5):
                        nc.tensor.matmul(py[:], R(w5_t[0:W5BLK[i][1], i, ms]),
                                         R(xcb[i][:, cs]),
                                         start=(i == 0), stop=(i == 4))
                    idx = s * 2 + ch
                    t1 = wk.tile([128, 512], F32, tag="trash", bufs=2)
                    nc.scalar.activation(t1[:], py[:], AF.Identity,
                                         accum_out=s1[:, m, idx:idx + 1])
                    t2 = wk.tile([128, 512], F32, tag="trash", bufs=2)
                    nc.scalar.activation(t2[:], py[:], AF.Square,
                                         accum_out=s2[:, m, idx:idx + 1])
        stat5 = wk.tile([128, 8, 2], F32, tag="stat5")
        for m in range(8):
            nc.vector.tensor_reduce(stat5[:, m, 0:1], s1[:, m, :], AX.X, ALU.add)
            nc.vector.tensor_reduce(stat5[:, m, 1:2], s2[:, m, :], AX.X, ALU.add)
        bin5 = dr.tile([128, 16], F32, name="bi5")
        bout5 = dr.tile([128, 16], F32, name="bo5")
        nc.sync.dma_start(bin5[:], stat5[:].rearrange("p a b -> p (a b)"))
        nc.gpsimd.collective_compute("AllReduce", ALU.add,
                                     replica_groups=[list(range(N_CORES))],
                                     ins=[bin5.opt()], outs=[bout5.opt()])
        statg5 = wk.tile([128, 8, 2], F32, tag="statg5")
        nc.sync.dma_start(statg5[:].rearrange("p a b -> p (a b)"), bout5[:])
        g5v = wk.tile([128, 8], F32, tag="g5v")
        b5v = wk.tile([128, 8], F32, tag="b5v")
        nc.sync.dma_start(g5v[:], g5_d.rearrange("(a p) -> p a", p=128))
        nc.sync.dma_start(b5v[:], b5_d.rearrange("(a p) -> p a", p=128))
        av5 = wk.tile([128, 8], F32, tag="av5")
        cv5 = wk.tile([128, 8], F32, tag="cv5")
        nav5 = wk.tile([128, 8], F32, tag="nav5")
        ncv5 = wk.tile([128, 8], F32, tag="ncv5")
        tv5 = wk.tile([128, 8, 4], F32, tag="tv5")
        for m in range(8):
            nc.vector.tensor_scalar(tv5[:, m, 0:1], statg5[:, m, 0:1], 1.0 / CNT_Y,
                                    None, ALU.mult)
            nc.vector.tensor_scalar(tv5[:, m, 1:2], statg5[:, m, 1:2], 1.0 / CNT_Y,
                                    None, ALU.mult)
            nc.vector.tensor_mul(tv5[:, m, 2:3], tv5[:, m, 0:1], tv5[:, m, 0:1])
            nc.vector.tensor_sub(tv5[:, m, 1:2], tv5[:, m, 1:2], tv5[:, m, 2:3])
            nc.vector.tensor_scalar(tv5[:, m, 1:2], tv5[:, m, 1:2], EPS, None, ALU.add)
            nc.vector.reciprocal(tv5[:, m, 2:3], tv5[:, m, 1:2])
            nc.scalar.activation(tv5[:, m, 3:4], tv5[:, m, 2:3], AF.Sqrt)
            nc.vector.tensor_mul(av5[:, m:m + 1], tv5[:, m, 3:4], g5v[:, m:m + 1])
            nc.vector.tensor_mul(tv5[:, m, 2:3], av5[:, m:m + 1], tv5[:, m, 0:1])
            nc.vector.tensor_sub(cv5[:, m:m + 1], b5v[:, m:m + 1], tv5[:, m, 2:3])
            nc.vector.tensor_scalar(nav5[:, m:m + 1], av5[:, m:m + 1], -1.0, None,
                                    ALU.mult)
            nc.vector.tensor_scalar(ncv5[:, m:m + 1], cv5[:, m:m + 1], -1.0, None,
                                    ALU.mult)

        # apply + pools (recompute y)
        gf = wk.tile([128, 16, 2], F32, tag="gf")  # blocks 0-7 max, 8-15 avg
        pacc = wk.tile([128, 8, 4], F32, tag="pacc")
        qacc = wk.tile([128, 8, 4], F32, tag="qacc")
        mxc = wk.tile([128, 8, 4], F32, tag="mxc")
        for s in range(BL):
            xcb = xc_rounded(s)
            for m in range(8):
                ms = slice(128 * m, 128 * (m + 1))
                for ch in range(2):
                    cs = slice(512 * ch, 512 * (ch + 1))
                    py = ps.tile([128, 512], F32, name=f"pp{s}{m}{ch}", tag="ps")
                    for i in range(5):
                        nc.tensor.matmul(py[:], R(w5_t[0:W5BLK[i][1], i, ms]),
                                         R(xcb[i][:, cs]),
                                         start=(i == 0), stop=(i == 4))
                    idx = s * 2 + ch
                    pr = wk.tile([128, 512], F32, tag="trash", bufs=2)
                    nc.scalar.activation(pr[:], py[:], AF.Relu,
                                         bias=cv5[:, m:m + 1], scale=av5[:, m:m + 1],
                                         accum_out=pacc[:, m, idx:idx + 1])
                    qr = wk.tile([128, 512], F32, tag="trash", bufs=2)
                    nc.scalar.activation(qr[:], py[:], AF.Relu,
                                         bias=ncv5[:, m:m + 1], scale=nav5[:, m:m + 1],
                                         accum_out=qacc[:, m, idx:idx + 1])
                    nc.vector.tensor_reduce(mxc[:, m, idx:idx + 1], py[:],
                                            AX.X, ALU.max)
            # per-sample pooling
            for m in range(8):
                i0, i1 = s * 2, s * 2 + 1
                mx = wk.tile([128, 1], F32, tag="mx5", bufs=2)
                nc.vector.tensor_max(mx[:], mxc[:, m, i0:i0 + 1], mxc[:, m, i1:i1 + 1])
                pm = wk.tile([128, 1], F32, tag="pm5", bufs=2)
                qm = wk.tile([128, 1], F32, tag="qm5", bufs=2)
                nc.scalar.activation(pm[:], mx[:], AF.Relu, bias=cv5[:, m:m + 1],
                                     scale=av5[:, m:m + 1])
                nc.scalar.activation(qm[:], mx[:], AF.Relu, bias=ncv5[:, m:m + 1],
                                     scale=nav5[:, m:m + 1])
                nc.vector.tensor_scalar(qm[:], qm[:], SLOPE, None, ALU.mult)
                nc.vector.tensor_sub(gf[:, m, s:s + 1], pm[:], qm[:])
                t = wk.tile([128, 2], F32, tag="tavg", bufs=2)
                nc.vector.tensor_add(t[:, 0:1], pacc[:, m, i0:i0 + 1],
                                     pacc[:, m, i1:i1 + 1])
                nc.vector.tensor_add(t[:, 1:2], qacc[:, m, i0:i0 + 1],
                                     qacc[:, m, i1:i1 + 1])
                nc.vector.tensor_scalar(t[:, 1:2], t[:, 1:2], SLOPE, None, ALU.mult)
                nc.vector.tensor_sub(t[:, 0:1], t[:, 0:1], t[:, 1:2])
                nc.vector.tensor_scalar(gf[:, 8 + m, s:s + 1], t[:, 0:1], 1.0 / N,
                                        None, ALU.mult)

        # ================= head =================
        def bn_head(h_sb, Mt_, gd, bd, ar_name):
            st = wk.tile([128, Mt_, 2], F32, tag=f"st_{ar_name}")
            for m in range(Mt_):
                nc.vector.tensor_add(st[:, m, 0:1], h_sb[:, m, 0:1], h_sb[:, m, 1:2])
                sq = wk.tile([128, 2], F32, tag=f"sq_{ar_name}", bufs=2)
                nc.scalar.activation(sq[:], h_sb[:, m, :], AF.Square)
                nc.vector.tensor_add(st[:, m, 1:2], sq[:, 0:1], sq[:, 1:2])
            bi = dr.tile([128, Mt_ * 2], F32, name=f"bih_{ar_name}")
            bo = dr.tile([128, Mt_ * 2], F32, name=f"boh_{ar_name}")
            nc.sync.dma_start(bi[:], st[:].rearrange("p a b -> p (a b)"))
            nc.gpsimd.collective_compute("AllReduce", ALU.add,
                                         replica_groups=[list(range(N_CORES))],
                                         ins=[bi.opt()], outs=[bo.opt()])
            sg = wk.tile([128, Mt_, 2], F32, tag=f"sg_{ar_name}")
            nc.sync.dma_start(sg[:].rearrange("p a b -> p (a b)"), bo[:])
            gv = wk.tile([128, Mt_], F32, tag=f"gv_{ar_name}")
            bv = wk.tile([128, Mt_], F32, tag=f"bv_{ar_name}")
            nc.sync.dma_start(gv[:], gd.rearrange("(a p) -> p a", p=128))
            nc.sync.dma_start(bv[:], bd.rearrange("(a p) -> p a", p=128))
            t = wk.tile([128, Mt_, 4], F32, tag=f"t_{ar_name}")
            for m in range(Mt_):
                nc.vector.tensor_scalar(t[:, m, 0:1], sg[:, m, 0:1], 1.0 / CNT_H,
                                        None, ALU.mult)
                nc.vector.tensor_scalar(t[:, m, 1:2], sg[:, m, 1:2], 1.0 / CNT_H,
                                        None, ALU.mult)
                nc.vector.tensor_mul(t[:, m, 2:3], t[:, m, 0:1], t[:, m, 0:1])
                nc.vector.tensor_sub(t[:, m, 1:2], t[:, m, 1:2], t[:, m, 2:3])
                nc.vector.tensor_scalar(t[:, m, 1:2], t[:, m, 1:2], EPS, None, ALU.add)
                nc.vector.reciprocal(t[:, m, 2:3], t[:, m, 1:2])
                nc.scalar.activation(t[:, m, 3:4], t[:, m, 2:3], AF.Sqrt)
                av_ = wk.tile([128, 1], F32, tag=f"av_{ar_name}", bufs=2)
                cv_ = wk.tile([128, 1], F32, tag=f"cv_{ar_name}", bufs=2)
                nc.vector.tensor_mul(av_[:], t[:, m, 3:4], gv[:, m:m + 1])
                nc.vector.tensor_mul(t[:, m, 2:3], av_[:], t[:, m, 0:1])
                nc.vector.tensor_sub(cv_[:], bv[:, m:m + 1], t[:, m, 2:3])
                nc.scalar.activation(h_sb[:, m, :], h_sb[:, m, :], AF.Relu,
                                     bias=cv_[:], scale=av_[:])

        fw1_t = wk.tile([128, 16, 256], F32, tag="bigB")
        for i in range(16):
            nc.sync.dma_start(fw1_t[:, i, :], fw1T_d[128 * i:128 * (i + 1), :])
        h1 = wk.tile([128, 2, 2], F32, tag="h1h")
        for m in range(2):
            ph = ps.tile([128, 2], F32, name=f"ph1{m}", tag="ps")
            for i in range(16):
                nc.tensor.matmul(ph[:], fw1_t[:, i, 128 * m:128 * (m + 1)],
                                 gf[:, i, :], start=(i == 0), stop=False)
            nc.tensor.matmul(ph[:], fb1row[:, 128 * m:128 * (m + 1)], ones2[:],
                             start=False, stop=True)
            nc.scalar.activation(h1[:, m, :], ph[:], AF.Copy)
        bn_head(h1, 2, fg1_d, fbb1_d, "h1")
        h2 = wk.tile([128, 2, 2], F32, tag="h2h")
        for m in range(2):
            ph = ps.tile([128, 2], F32, name=f"ph2{m}", tag="ps")
            for i in range(2):
                nc.tensor.matmul(ph[:], fw2_sb[i][:, 128 * m:128 * (m + 1)],
                                 h1[:, i, :], start=(i == 0), stop=False)
            nc.tensor.matmul(ph[:], fb2row[:, 128 * m:128 * (m + 1)], ones2[:],
                             start=False, stop=True)
            nc.scalar.activation(h2[:, m, :], ph[:], AF.Copy)
        bn_head(h2, 2, fg2_d, fbb2_d, "h2")
        lg = wk.tile([2, CLS], F32, tag="lg")
        for ch, (c0, cw) in enumerate([(0, 512), (512, CLS - 512)]):
            pl = ps.tile([2, 512], F32, name=f"pl{ch}", tag="ps")
            for i in range(2):
                nc.tensor.matmul(pl[:, 0:cw], h2[:, i, :], fw3_sb[i][:, c0:c0 + cw],
                                 start=(i == 0), stop=False)
            nc.tensor.matmul(pl[:, 0:cw], ones2[:], fb3row[:, c0:c0 + cw],
                             start=False, stop=True)
            nc.scalar.activation(lg[:, c0:c0 + cw], pl[:, 0:cw], AF.Copy)
        mxl = wk.tile([2, 4], F32, tag="mxl")
        nc.vector.tensor_reduce(mxl[:, 0:1], lg[:], AX.X, ALU.max)
        nc.vector.tensor_scalar(mxl[:, 1:2], mxl[:, 0:1], -1.0, None, ALU.mult)
        ex = wk.tile([2, CLS], F32, tag="exh")
        nc.scalar.activation(ex[:], lg[:], AF.Exp, bias=mxl[:, 1:2],
                             accum_out=mxl[:, 2:3])
        nc.scalar.activation(mxl[:, 3:4], mxl[:, 2:3], AF.Ln)
        nc.vector.tensor_add(mxl[:, 3:4], mxl[:, 3:4], mxl[:, 0:1])
        nc.vector.tensor_scalar(mxl[:, 3:4], mxl[:, 3:4], -1.0, None, ALU.mult)
        outt = wk.tile([2, CLS], F32, tag="outh")
        nc.scalar.activation(outt[:], lg[:], AF.Identity, bias=mxl[:, 3:4])
        nc.sync.dma_start(out_d[:], outt[:])

    nc.compile()
    return nc


_NC_CACHE = None


def build_in_maps(inputs):
    pts = np.asarray(inputs["pts"], dtype=np.float32)

    def T(x):
        return np.ascontiguousarray(np.asarray(x, dtype=np.float32).T)

    base = {}
    for li in range(4):
        C = LAYERS[li][0]
        w = np.asarray(inputs[f"w{li + 1}"], dtype=np.float32)
        base[f"wn{li}"] = np.ascontiguousarray(w[:, :C].T)
        base[f"wd{li}"] = np.ascontiguousarray((w[:, C:] - w[:, :C]).T)
        base[f"g{li}"] = np.asarray(inputs[f"g{li + 1}"], dtype=np.float32)
        base[f"b{li}"] = np.asarray(inputs[f"b{li + 1}"], dtype=np.float32)
    base["w5T"] = T(inputs["w5"])
    base["g5"] = np.asarray(inputs["g5"], dtype=np.float32)
    base["b5"] = np.asarray(inputs["b5"], dtype=np.float32)
    base["fw1T"] = np.vstack([T(inputs["fw1"]),
                              np.asarray(inputs["fb1"], np.float32)[None, :]])
    base["fg1"] = np.asarray(inputs["fg1"], np.float32)
    base["fbb1"] = np.asarray(inputs["fbb1"], np.float32)
    base["fw2T"] = np.vstack([T(inputs["fw2"]),
                              np.asarray(inputs["fb2"], np.float32)[None, :]])
    base["fg2"] = np.asarray(inputs["fg2"], np.float32)
    base["fbb2"] = np.asarray(inputs["fbb2"], np.float32)
    base["fw3T"] = np.vstack([T(inputs["fw3"]),
                              np.asarray(inputs["fb3"], np.float32)[None, :]])

    in_maps = []
    for c in range(N_CORES):
        m = dict(base)
        shard = pts[c * BL:(c + 1) * BL]
        m["ptsT"] = np.ascontiguousarray(shard.transpose(0, 2, 1))
        in_maps.append(m)
    return in_maps


def kernel(**inputs):
    global _NC_CACHE
    if _NC_CACHE is None:
        _NC_CACHE = build()
    nc = _NC_CACHE
    in_maps = build_in_maps(inputs)
    res = bass_utils.run_bass_kernel_spmd(nc, in_maps, core_ids=list(range(N_CORES)))
    out = np.concatenate([res.results[c]["out"] for c in range(N_CORES)], axis=0)
    return out.astype(np.float32)


# revision 20
# speedup vs baseline: 1.4894x; 1.4894x over previous
"""DGCNN-ReID forward pass on 8 Trainium2 NeuronCores (Bass/Tile).

Data-parallel over batch (16 samples -> 2 per core). Per EdgeConv layer:
the kNN affinity matrix is built on the TensorEngine, exact top-20
neighbor indices are extracted with DVE max8/max_index/match_replace
rounds (stable-argsort semantics, tie-safe), and neighbor features are
fetched with GPSIMD indirect_copy gathers from a feature-major u table,
followed by a max-reduce over the 20 neighbors. Training-mode BatchNorm
statistics: layers 1-2 accumulate E[y], E[y^2] directly from the gathered
edge values (avoids catastrophic cancellation of the sum-decomposition at
small feature scales); layers 3-4 use one cumulative top-k mask matmul
per sample (bf16) for the neighbor sums. Stats are AllReduced across the
8 cores.
"""
import sys
sys.path.insert(0, '/opt/trn_rl_repo')
import numpy as np
from contextlib import ExitStack

import concourse.bass as bass
import concourse.tile as tile
from concourse import bacc, mybir
from concourse import bass_utils

dt = mybir.dt
F32 = dt.float32
F32R = dt.float32r
BF16 = dt.bfloat16
U16 = dt.uint16
AF = mybir.ActivationFunctionType
ALU = mybir.AluOpType
AX = mybir.AxisListType

N_CORES = 8
B, BL, N, K = 16, 2, 1024, 20
NQ = N // 128
EPS = 1e-5
SLOPE = 0.2
EMB = 1024
CLS = 751
LAYERS = [(3, 64), (64, 64), (64, 128), (128, 256)]
DIRECT = [True, True, False, False]
CNT_EDGE = float(B * N * K)
CNT_Y = float(B * N)
CNT_H = float(B)
NEG = -1e30


def R(ap):
    return ap.bitcast(F32R)


def kb_blocks(C):
    out = []
    c = 0
    while c < C:
        out.append((c, min(128, C - c)))
        c += 128
    return out


def build():
    nc = bacc.Bacc("TRN2", target_bir_lowering=False, debug=False,
                   enable_asserts=True, num_devices=N_CORES)

    ptsT = nc.dram_tensor("ptsT", [BL, 3, N], F32, kind="ExternalInput").ap()
    wn_d, wd_d, gm_d, bt_d = [], [], [], []
    for li, (C, O) in enumerate(LAYERS):
        wn_d.append(nc.dram_tensor(f"wn{li}", [C, O], F32, kind="ExternalInput").ap())
        wd_d.append(nc.dram_tensor(f"wd{li}", [C, O], F32, kind="ExternalInput").ap())
        gm_d.append(nc.dram_tensor(f"g{li}", [O], F32, kind="ExternalInput").ap())
        bt_d.append(nc.dram_tensor(f"b{li}", [O], F32, kind="ExternalInput").ap())
    w5T_d = nc.dram_tensor("w5T", [512, EMB], F32, kind="ExternalInput").ap()
    g5_d = nc.dram_tensor("g5", [EMB], F32, kind="ExternalInput").ap()
    b5_d = nc.dram_tensor("b5", [EMB], F32, kind="ExternalInput").ap()
    fw1T_d = nc.dram_tensor("fw1T", [2 * EMB + 1, 256], F32, kind="ExternalInput").ap()
    fg1_d = nc.dram_tensor("fg1", [256], F32, kind="ExternalInput").ap()
    fbb1_d = nc.dram_tensor("fbb1", [256], F32, kind="ExternalInput").ap()
    fw2T_d = nc.dram_tensor("fw2T", [257, 256], F32, kind="ExternalInput").ap()
    fg2_d = nc.dram_tensor("fg2", [256], F32, kind="ExternalInput").ap()
    fbb2_d = nc.dram_tensor("fbb2", [256], F32, kind="ExternalInput").ap()
    fw3T_d = nc.dram_tensor("fw3T", [257, CLS], F32, kind="ExternalInput").ap()
    out_d = nc.dram_tensor("out", [BL, CLS], F32, kind="ExternalOutput").ap()

    with tile.TileContext(nc) as tc, ExitStack() as ctx:
        sb = ctx.enter_context(tc.tile_pool(name="sb", bufs=1))
        wk = ctx.enter_context(tc.tile_pool(name="wk", bufs=1))
        ps = ctx.enter_context(tc.tile_pool(name="ps", bufs=8, space="PSUM"))
        dr = ctx.enter_context(tc.tile_pool(name="dr", bufs=1, space="DRAM"))

        ones_row = sb.tile([1, N], F32)
        nc.vector.memset(ones_row[:], 1.0)
        ones2 = sb.tile([1, 2], F32)
        nc.vector.memset(ones2[:], 1.0)
        iot = sb.tile([128, 128], F32)
        nc.gpsimd.iota(iot[:], pattern=[[1, 128]], base=0, channel_multiplier=-1,
                       allow_small_or_imprecise_dtypes=True)
        identb = sb.tile([128, 128], BF16)
        nc.vector.tensor_scalar(identb[:], iot[:], 0.0, None, ALU.is_equal)

        # ---- load weights ----
        wn_sb, wd_sb = [], []
        for li, (C, O) in enumerate(LAYERS):
            t1 = sb.tile([C, O], F32, name=f"wn{li}")
            nc.sync.dma_start(t1[:], wn_d[li][:, :])
            wn_sb.append(t1)
            t2 = sb.tile([C, O], F32, name=f"wd{li}")
            nc.sync.dma_start(t2[:], wd_d[li][:, :])
            wd_sb.append(t2)
        W5BLK = [(0, 64), (64, 64), (128, 128), (256, 128), (384, 128)]
        fb1row = sb.tile([1, 256], F32)
        nc.sync.dma_start(fb1row[:], fw1T_d[2 * EMB:2 * EMB + 1, :])
        fw2_sb = []
        for i, (c0, cb) in enumerate(kb_blocks(256)):
            t = sb.tile([cb, 256], F32, name=f"fw2_{i}")
            nc.sync.dma_start(t[:], fw2T_d[c0:c0 + cb, :])
            fw2_sb.append(t)
        fb2row = sb.tile([1, 256], F32)
        nc.sync.dma_start(fb2row[:], fw2T_d[256:257, :])
        fw3_sb = []
        for i, (c0, cb) in enumerate(kb_blocks(256)):
            t = sb.tile([cb, CLS], F32, name=f"fw3_{i}")
            nc.sync.dma_start(t[:], fw3T_d[c0:c0 + cb, :])
            fw3_sb.append(t)
        fb3row = sb.tile([1, CLS], F32)
        nc.sync.dma_start(fb3row[:], fw3T_d[256:257, :])

        # persistent x_l.T feature tiles (layer 0 input = pts)
        xs = [[sb.tile([LAYERS[li][0], N], F32, name=f"x{li}_{s}")
               for s in range(BL)] for li in range(4)]
        x4 = [[sb.tile([128, N], F32, name=f"x4_{s}_{i}") for i in range(2)]
              for s in range(BL)]
        for s in range(BL):
            nc.sync.dma_start(xs[0][s][:], ptsT[s])

        def xnext_tiles(li, s):
            # output z.T destination tiles for layer li: list of (tile, ob)
            if li + 1 < 4:
                return [(xs[li + 1][s], LAYERS[li][1])]
            return [(x4[s][0], 128), (x4[s][1], 128)]

        # ================= edge layers =================
        for li, (C, O) in enumerate(LAYERS):
            Mt = (O + 127) // 128
            direct = DIRECT[li]
            if direct:
                syp = wk.tile([128, BL * 4, NQ], F32, tag="syp")
                sy2p = wk.tile([128, BL * 4], F32, tag="sy2p")
            else:
                sup = wk.tile([128, Mt, BL * NQ], F32, tag="syp")
                sqp = wk.tile([128, Mt, BL * NQ], F32, tag="sy2p")
                crp = wk.tile([128, Mt, BL * NQ], F32, tag="crp")
                scp = wk.tile([128, Mt, BL * 2], F32, tag="scp")
                scqp = wk.tile([128, Mt, BL * 2], F32, tag="scqp")

            for s in range(BL):
                xb = xs[li][s]
                xsq = wk.tile([C, N], F32, tag="xsq", bufs=2)
                nc.scalar.activation(xsq[:], xb[:], AF.Square)
                onesc = wk.tile([C, 1], F32, tag="onesc", bufs=2)
                nc.vector.memset(onesc[:], 1.0)
                xxneg = wk.tile([1, N], F32, tag="xxneg", bufs=2)
                for ch in range(2):
                    cs = slice(512 * ch, 512 * (ch + 1))
                    pxx = ps.tile([1, 512], F32, name=f"pxx{li}{s}{ch}", tag="ps")
                    nc.tensor.matmul(pxx[:], onesc[:], xsq[:, cs], start=True,
                                     stop=True)
                    nc.scalar.activation(xxneg[:, cs], pxx[:], AF.Identity,
                                         scale=-0.5)

                # u.T (f32 gather table)
                uTs = []
                for m in range(Mt):
                    ob = min(128, O - 128 * m)
                    ut = wk.tile([128, N], F32, tag="uT0", bufs=2)
                    for ch in range(2):
                        cs = slice(512 * ch, 512 * (ch + 1))
                        pu = ps.tile([ob, 512], F32, name=f"pu{li}{s}{m}{ch}",
                                     tag="ps")
                        nc.tensor.matmul(pu[:], wn_sb[li][:, 128 * m:128 * m + ob],
                                         xb[:, cs], start=True, stop=True)
                        nc.scalar.activation(ut[0:ob, cs], pu[:], AF.Copy)
                    uTs.append(ut)
                if O == 64:
                    nc.sync.dma_start(uTs[0][64:128, :], uTs[0][0:64, :])

                # c.T = ((wc-wn).T x).T
                cT = wk.tile([128, Mt, N], F32, tag="cT")
                for m in range(Mt):
                    ob = min(128, O - 128 * m)
                    for ch in range(2):
                        cs = slice(512 * ch, 512 * (ch + 1))
                        pc = ps.tile([ob, 512], F32, name=f"pc{li}{s}{m}{ch}",
                                     tag="ps")
                        nc.tensor.matmul(pc[:], wd_sb[li][:, 128 * m:128 * m + ob],
                                         xb[:, cs], start=True, stop=True)
                        if direct:
                            nc.scalar.activation(cT[0:ob, m, cs], pc[:], AF.Copy)
                        else:
                            nc.scalar.activation(cT[0:ob, m, cs], pc[:],
                                                 AF.Identity,
                                                 accum_out=scp[0:ob, m,
                                                              2 * s + ch:2 * s + ch + 1])
                            tr = wk.tile([128, 512], F32, tag="trash", bufs=2)
                            nc.scalar.activation(tr[0:ob, :], cT[0:ob, m, cs],
                                                 AF.Square,
                                                 accum_out=scqp[0:ob, m,
                                                               2 * s + ch:2 * s + ch + 1])
                cd = None
                if direct:
                    # rows 0-63: c.T; rows 64-127: c.T shifted left by 128 cols
                    cd = wk.tile([128, N], F32, tag="cdup")
                    nc.sync.dma_start(cd[0:64, :], cT[0:64, 0, :])
                    nc.sync.dma_start(cd[64:128, 0:N - 128], cT[0:64, 0, 128:N])

                # point-major u and u^2 (bf16) for mask-stat matmuls
                if not direct:
                    upm = wk.tile([128, NQ, O], BF16, tag="upm")
                    usq = wk.tile([128, NQ, O], BF16, tag="usq")
                    for ib in range(NQ):
                        pp = ps.tile([128, O], F32, name=f"pp{li}{s}{ib}", tag="ps")
                        nc.tensor.matmul(pp[:], xb[:, 128 * ib:128 * (ib + 1)],
                                         wn_sb[li][:, :], start=True, stop=True)
                        nc.scalar.activation(upm[:, ib, :], pp[:], AF.Copy)
                        nc.scalar.activation(usq[:, ib, :], upm[:, ib, :], AF.Square)

                # ---- affinity + top-20 indices per dest block ----
                i24 = wk.tile([128, NQ, 24], U16, tag="i24", bufs=2)
                for q in range(NQ):
                    qs = slice(128 * q, 128 * (q + 1))
                    sq_ = wk.tile([128, N], F32, tag="sq", bufs=3)
                    for ch in range(2):
                        cs = slice(512 * ch, 512 * (ch + 1))
                        pss = ps.tile([128, 512], F32, name=f"pss{li}{s}{q}{ch}",
                                      tag="ps")
                        nc.tensor.matmul(pss[:], xb[:, qs], xb[:, cs],
                                         start=True, stop=False)
                        nc.tensor.matmul(pss[:], ones_row[:, qs], xxneg[:, cs],
                                         start=False, stop=True)
                        nc.scalar.activation(sq_[:, cs], pss[:], AF.Copy)
                    v24 = wk.tile([128, 24], F32, tag="v24", bufs=2)
                    nc.vector.max(v24[:, 0:8], sq_[:])
                    nc.vector.max_index(i24[:, q, 0:8], v24[:, 0:8], sq_[:])
                    sq2 = wk.tile([128, N], F32, tag="sq", bufs=3)
                    nc.vector.match_replace(sq2[:], v24[:, 0:8], sq_[:], NEG)
                    nc.vector.max(v24[:, 8:16], sq2[:])
                    nc.vector.max_index(i24[:, q, 8:16], v24[:, 8:16], sq2[:])
                    sq3 = wk.tile([128, N], F32, tag="sq", bufs=3)
                    nc.vector.match_replace(sq3[:], v24[:, 8:16], sq2[:], NEG)
                    nc.vector.max(v24[:, 16:24], sq3[:])
                    nc.vector.max_index(i24[:, q, 16:24], v24[:, 16:24], sq3[:])
                    if not direct:
                        mq = wk.tile([128, N], BF16, tag="mskq")
                        nc.vector.tensor_scalar(mq[:], sq_[:], v24[:, 19:20],
                                                None, ALU.is_ge)
                        mskq_t = wk.tile([128, NQ, 128], BF16, tag="mskT", bufs=2)
                        for jb in range(NQ):
                            pt = ps.tile([128, 128], BF16,
                                         name=f"pt{li}{s}{q}{jb}", tag="ps")
                            nc.tensor.transpose(pt[:], mq[:, 128 * jb:128 * (jb + 1)],
                                                identb[:])
                            nc.scalar.activation(mskq_t[:, jb, :], pt[:], AF.Copy)
                        for m in range(Mt):
                            ob = min(128, O - 128 * m)
                            ms = slice(128 * m, 128 * m + ob)
                            pS = ps.tile([128, 128], F32,
                                         name=f"pS{li}{s}{q}{m}", tag="ps")
                            pQ = ps.tile([128, 128], F32,
                                         name=f"pQ{li}{s}{q}{m}", tag="ps")
                            for jb in range(NQ):
                                nc.tensor.matmul(pS[0:ob, :], upm[:, jb, ms],
                                                 mskq_t[:, jb, :],
                                                 start=(jb == 0), stop=(jb == NQ - 1))
                                nc.tensor.matmul(pQ[0:ob, :], usq[:, jb, ms],
                                                 mskq_t[:, jb, :],
                                                 start=(jb == 0), stop=(jb == NQ - 1))
                            ci = s * NQ + q
                            tr = wk.tile([128, 512], F32, tag="trash", bufs=2)
                            nc.scalar.activation(tr[0:ob, 0:128], pS[0:ob, :],
                                                 AF.Identity,
                                                 accum_out=sup[0:ob, m, ci:ci + 1])
                            tr2 = wk.tile([128, 512], F32, tag="trash", bufs=2)
                            nc.scalar.activation(tr2[0:ob, 0:128], pQ[0:ob, :],
                                                 AF.Identity,
                                                 accum_out=sqp[0:ob, m, ci:ci + 1])
                            trd = wk.tile([128, 512], F32, tag="trash", bufs=2)
                            nc.vector.scalar_tensor_tensor(
                                trd[0:ob, 0:128], pS[0:ob, :], 1.0,
                                cT[0:ob, m, qs], ALU.mult, ALU.mult,
                                accum_out=crp[0:ob, m, ci:ci + 1])

                # ---- index wrap for gpsimd gathers (via DRAM bounce) ----
                # list order t = 16*(j*20 + k) + p for dest i = 16*j + p, so
                # both bounce DMAs move contiguous 20/160-element u16 runs.
                # G comes out as [ch, j(8), k(20), p(16)].
                dram_w = dr.tile([NQ, 16, 160], U16, name=f"idxd{li}{s}")
                for j in range(NQ):
                    d1 = bass.AP(dram_w.tensor, dram_w[:].offset + 20 * j,
                                 [[160, 16], [2560, NQ], [1, 20]])
                    nc.sync.dma_start(d1, i24[16 * j:16 * (j + 1), :, 0:20])
                if O == 64:
                    idxw = wk.tile([128, 4, 192], U16, tag="idxw")
                    for g in range(8):
                        srcg = bass.AP(dram_w.tensor,
                                       dram_w[:].offset + 2560 * (g // 4),
                                       [[160, 16], [5120, 4], [1, 160]])
                        dslc = idxw[16 * g:16 * (g + 1), :, :]
                        dstg = bass.AP(idxw.tensor, dslc.offset,
                                       [dslc.ap[0], [192, 4], [1, 160]])
                        nc.sync.dma_start(dstg, srcg)
                else:
                    idxw = wk.tile([128, NQ, 192], U16, tag="idxw")
                    for g in range(8):
                        srcg = bass.AP(dram_w.tensor, dram_w[:].offset,
                                       [[160, 16], [2560, NQ], [1, 160]])
                        dslc = idxw[16 * g:16 * (g + 1), :, :]
                        dstg = bass.AP(idxw.tensor, dslc.offset,
                                       [dslc.ap[0], [192, NQ], [1, 160]])
                        nc.sync.dma_start(dstg, srcg)

                def pool_kmax(G, ob):
                    # max over k: G is [128, j(8), k(20), p(16)]; transposed AP
                    # view puts k innermost; out columns are dest-ordered.
                    Gv = G[:].rearrange("p (j k i) -> p j i k", k=20, i=16)
                    zt = wk.tile([128, 128], F32, tag="zt", bufs=2)
                    nc.vector.tensor_reduce(
                        zt[0:ob, :].rearrange("p (j i) -> p j i", i=16),
                        Gv[0:ob], AX.X, ALU.max)
                    return zt

                if direct:
                    for t in range(4):
                        G = wk.tile([128, 2560], F32, tag="G", bufs=2)
                        for a, b in [(0, 64), (64, 128), (128, 160)]:
                            nc.gpsimd.indirect_copy(
                                G[:, 16 * a:16 * b].rearrange("p (i o) -> p i o", o=1),
                                uTs[0][:], idxw[:, t, a:b], True)
                        cds = cd[:, 256 * t:256 * t + 128]
                        tG = wk.tile([128, 2560], F32, tag="tG")
                        tGv = tG[:].rearrange("p (j k i) -> p j k i", k=20, i=16)
                        Gvv = G[:].rearrange("p (j k i) -> p j k i", k=20, i=16)
                        for j in range(NQ):
                            cb = bass.AP(cd.tensor, cds.offset + 16 * j,
                                         [cds.ap[0], [0, 20], [1, 16]])
                            nc.vector.scalar_tensor_tensor(
                                tGv[:, j], Gvv[:, j], 0.0, cb, ALU.add, ALU.add,
                                accum_out=syp[:, s * 4 + t, j:j + 1])
                        nc.scalar.activation(tG[:], tG[:], AF.Square,
                                             accum_out=sy2p[:, s * 4 + t:s * 4 + t + 1])
                        zt = pool_kmax(G, 128)
                        dst, _ = xnext_tiles(li, s)[0]
                        nc.vector.tensor_add(dst[0:64, 256 * t:256 * t + 128],
                                             zt[0:64, :], cd[0:64, 256 * t:256 * t + 128])
                        zhi = wk.tile([128, 128], F32, tag="zt", bufs=2)
                        nc.vector.tensor_add(zhi[64:128, :], zt[64:128, :],
                                             cd[64:128, 256 * t:256 * t + 128])
                        nc.sync.dma_start(dst[0:64, 256 * t + 128:256 * t + 256],
                                          zhi[64:128, :])
                else:
                    for q in range(NQ):
                        for m in range(Mt):
                            ob = min(128, O - 128 * m)
                            G = wk.tile([128, 2560], F32, tag="G", bufs=2)
                            for a, b in [(0, 64), (64, 128), (128, 160)]:
                                nc.gpsimd.indirect_copy(
                                    G[:, 16 * a:16 * b].rearrange("p (i o) -> p i o", o=1),
                                    uTs[m][:], idxw[:, q, a:b], True)
                            zt = pool_kmax(G, ob)
                            dst, _ = xnext_tiles(li, s)[m]
                            nc.vector.tensor_add(dst[0:ob, 128 * q:128 * (q + 1)],
                                                 zt[0:ob, :],
                                                 cT[0:ob, m, 128 * q:128 * (q + 1)])

            # ---- finalize stats ----
            stat = wk.tile([128, Mt, 2], F32, tag="stat")
            nc.vector.memset(stat[:].rearrange("p a b -> p (a b)"), 0.0)
            if direct:
                sypf = syp[:].rearrange("p a b -> p (a b)")
                fold1 = wk.tile([64, BL * 4 * NQ], F32, tag="fold1")
                nc.sync.dma_start(fold1[:], sypf[64:128, :])
                fold2 = wk.tile([64, BL * 4], F32, tag="fold2")
                nc.sync.dma_start(fold2[:], sy2p[64:128, :])
                nc.vector.tensor_add(sypf[0:64, :], sypf[0:64, :], fold1[:])
                nc.vector.tensor_add(sy2p[0:64, :], sy2p[0:64, :], fold2[:])
                nc.vector.tensor_reduce(stat[0:64, 0, 0:1], sypf[0:64, :], AX.X,
                                        ALU.add)
                nc.vector.tensor_reduce(stat[0:64, 0, 1:2], sy2p[0:64, :], AX.X,
                                        ALU.add)
            else:
                red = wk.tile([128, Mt, 5], F32, tag="red")
                for m in range(Mt):
                    ob = min(128, O - 128 * m)
                    nc.vector.tensor_reduce(red[0:ob, m, 0:1], sup[0:ob, m, :],
                                            AX.X, ALU.add)
                    nc.vector.tensor_reduce(red[0:ob, m, 1:2], sqp[0:ob, m, :],
                                            AX.X, ALU.add)
                    nc.vector.tensor_reduce(red[0:ob, m, 2:3], crp[0:ob, m, :],
                                            AX.X, ALU.add)
                    nc.vector.tensor_reduce(red[0:ob, m, 3:4], scp[0:ob, m, :],
                                            AX.X, ALU.add)
                    nc.vector.tensor_reduce(red[0:ob, m, 4:5], scqp[0:ob, m, :],
                                            AX.X, ALU.add)
                    # Sy = Su + K*Sc
                    nc.vector.tensor_scalar(stat[0:ob, m, 0:1], red[0:ob, m, 3:4],
                                            float(K), None, ALU.mult)
                    nc.vector.tensor_add(stat[0:ob, m, 0:1], stat[0:ob, m, 0:1],
                                         red[0:ob, m, 0:1])
                    # Sy2 = Sq + 2*cross + K*Scq
                    nc.vector.tensor_scalar(stat[0:ob, m, 1:2], red[0:ob, m, 2:3],
                                            2.0, None, ALU.mult)
                    nc.vector.tensor_add(stat[0:ob, m, 1:2], stat[0:ob, m, 1:2],
                                         red[0:ob, m, 1:2])
                    tk = wk.tile([128, 1], F32, tag="tk", bufs=2)
                    nc.vector.tensor_scalar(tk[0:ob, :], red[0:ob, m, 4:5],
                                            float(K), None, ALU.mult)
                    nc.vector.tensor_add(stat[0:ob, m, 1:2], stat[0:ob, m, 1:2],
                                         tk[0:ob, :])

            # ---- AllReduce + BN coefs + apply ----
            bin_ = dr.tile([128, Mt * 2], F32, name=f"bi{li}")
            bout = dr.tile([128, Mt * 2], F32, name=f"bo{li}")
            nc.sync.dma_start(bin_[:], stat[:].rearrange("p a b -> p (a b)"))
            nc.gpsimd.collective_compute("AllReduce", ALU.add,
                                         replica_groups=[list(range(N_CORES))],
                                         ins=[bin_.opt()], outs=[bout.opt()])
            statg = wk.tile([128, Mt, 2], F32, tag="statg")
            nc.sync.dma_start(statg[:].rearrange("p a b -> p (a b)"), bout[:])
            gamv = wk.tile([128, Mt], F32, tag="gamv")
            betv = wk.tile([128, Mt], F32, tag="betv")
            for m in range(Mt):
                ob = min(128, O - 128 * m)
                nc.sync.dma_start(gamv[0:ob, m:m + 1],
                                  gm_d[li][128 * m:128 * m + ob].rearrange("(p a) -> p a", a=1))
                nc.sync.dma_start(betv[0:ob, m:m + 1],
                                  bt_d[li][128 * m:128 * m + ob].rearrange("(p a) -> p a", a=1))
            av = wk.tile([128, Mt], F32, tag="av")
            cv = wk.tile([128, Mt], F32, tag="cv")
            nav = wk.tile([128, Mt], F32, tag="nav")
            ncv = wk.tile([128, Mt], F32, tag="ncv")
            tv = wk.tile([128, Mt, 4], F32, tag="tv")
            for m in range(Mt):
                ob = min(128, O - 128 * m)
                nc.vector.tensor_scalar(tv[0:ob, m, 0:1], statg[0:ob, m, 0:1],
                                        1.0 / CNT_EDGE, None, ALU.mult)
                nc.vector.tensor_scalar(tv[0:ob, m, 1:2], statg[0:ob, m, 1:2],
                                        1.0 / CNT_EDGE, None, ALU.mult)
                nc.vector.tensor_mul(tv[0:ob, m, 2:3], tv[0:ob, m, 0:1],
                                     tv[0:ob, m, 0:1])
                nc.vector.tensor_sub(tv[0:ob, m, 1:2], tv[0:ob, m, 1:2],
                                     tv[0:ob, m, 2:3])
                nc.vector.tensor_scalar(tv[0:ob, m, 1:2], tv[0:ob, m, 1:2], EPS,
                                        None, ALU.add)
                nc.vector.reciprocal(tv[0:ob, m, 2:3], tv[0:ob, m, 1:2])
                nc.scalar.activation(tv[0:ob, m, 3:4], tv[0:ob, m, 2:3], AF.Sqrt)
                nc.vector.tensor_mul(av[0:ob, m:m + 1], tv[0:ob, m, 3:4],
                                     gamv[0:ob, m:m + 1])
                nc.vector.tensor_mul(tv[0:ob, m, 2:3], av[0:ob, m:m + 1],
                                     tv[0:ob, m, 0:1])
                nc.vector.tensor_sub(cv[0:ob, m:m + 1], betv[0:ob, m:m + 1],
                                     tv[0:ob, m, 2:3])
                nc.vector.tensor_scalar(nav[0:ob, m:m + 1], av[0:ob, m:m + 1], -1.0,
                                        None, ALU.mult)
                nc.vector.tensor_scalar(ncv[0:ob, m:m + 1], cv[0:ob, m:m + 1], -1.0,
                                        None, ALU.mult)
            for s in range(BL):
                for m in range(Mt):
                    ob = min(128, O - 128 * m)
                    dst, _ = xnext_tiles(li, s)[m]
                    p_s = wk.tile([128, N], F32, tag="sq", bufs=3)
                    q_s = wk.tile([128, N], F32, tag="sq", bufs=3)
                    nc.scalar.activation(p_s[0:ob, :], dst[0:ob, :], AF.Relu,
                                         bias=cv[0:ob, m:m + 1],
                                         scale=av[0:ob, m:m + 1])
                    nc.scalar.activation(q_s[0:ob, :], dst[0:ob, :], AF.Relu,
                                         bias=ncv[0:ob, m:m + 1],
                                         scale=nav[0:ob, m:m + 1])
                    nc.vector.tensor_scalar(q_s[0:ob, :], q_s[0:ob, :], SLOPE,
                                            None, ALU.mult)
                    nc.vector.tensor_sub(dst[0:ob, :], p_s[0:ob, :], q_s[0:ob, :])

        # ================= conv5 + pooling =================
        w5_t = wk.tile([128, 5, EMB], F32R, tag="bigB", name="w5_t")
        for i, (c0, cb) in enumerate(W5BLK):
            w5_s = wk.tile([128, EMB], F32, tag="xsq", bufs=2)
            nc.sync.dma_start(w5_s[0:cb, :], w5T_d[c0:c0 + cb, :])
            nc.scalar.activation(w5_t[0:cb, i, :], w5_s[0:cb, :], AF.Copy)

        def xc_blocks(s):
            return [xs[1][s], xs[2][s], xs[3][s], x4[s][0], x4[s][1]]

        XCTAGS = [("cdup", 1), ("xsq", 2), ("uT0", 2), ("G", 2), ("uT0", 2)]

        def xc_rounded(s):
            out = []
            for i, t in enumerate(xc_blocks(s)):
                cb = t.shape[0]
                tg, bf = XCTAGS[i]
                rt = wk.tile([cb, N], F32R, name=f"xcr{s}_{i}", tag=tg, bufs=bf)
                nc.scalar.activation(rt[:], t[:], AF.Copy)
                out.append(rt)
            return out

        s1 = wk.tile([128, 8, 4], F32, tag="s1c")
        s2 = wk.tile([128, 8, 4], F32, tag="s2c")
        for s in range(BL):
            xcb = xc_rounded(s)
            for m in range(8):
                ms = slice(128 * m, 128 * (m + 1))
                for ch in range(2):
                    cs = slice(512 * ch, 512 * (ch + 1))
                    py = ps.tile([128, 512], F32, name=f"pw{s}{m}{ch}", tag="ps")
                    for i in range(5):
                        nc.tensor.matmul(py[:], R(w5_t[0:W5BLK[i][1], i, ms]),
                                         R(xcb[i][:, cs]),
                                         start=(i == 0), stop=(i == 4))
                    idx = s * 2 + ch
                    t1 = wk.tile([128, 512], F32, tag="trash", bufs=2)
                    nc.scalar.activation(t1[:], py[:], AF.Identity,
                                         accum_out=s1[:, m, idx:idx + 1])
                    t2 = wk.tile([128, 512], F32, tag="trash", bufs=2)
                    nc.scalar.activation(t2[:], py[:], AF.Square,
                                         accum_out=s2[:, m, idx:idx + 1])
        stat5 = wk.tile([128, 8, 2], F32, tag="stat5")
        for m in range(8):
            nc.vector.tensor_reduce(stat5[:, m, 0:1], s1[:, m, :], AX.X, ALU.add)
            nc.vector.tensor_reduce(stat5[:, m, 1:2], s2[:, m, :], AX.X, ALU.add)
        bin5 = dr.tile([128, 16], F32, name="bi5")
        bout5 = dr.tile([128, 16], F32, name="bo5")
        nc.sync.dma_start(bin5[:], stat5[:].rearrange("p a b -> p (a b)"))
        nc.gpsimd.collective_compute("AllReduce", ALU.add,
                                     replica_groups=[list(range(N_CORES))],
                                     ins=[bin5.opt()], outs=[bout5.opt()])
        statg5 = wk.tile([128, 8, 2], F32, tag="statg5")
        nc.sync.dma_start(statg5[:].rearrange("p a b -> p (a b)"), bout5[:])
        g5v = wk.tile([128, 8], F32, tag="g5v")
        b5v = wk.tile([128, 8], F32, tag="b5v")
        nc.sync.dma_start(g5v[:], g5_d.rearrange("(a p) -> p a", p=128))
        nc.sync.dma_start(b5v[:], b5_d.rearrange("(a p) -> p a", p=128))
        av5 = wk.tile([128, 8], F32, tag="av5")
        cv5 = wk.tile([128, 8], F32, tag="cv5")
        nav5 = wk.tile([128, 8], F32, tag="nav5")
        ncv5 = wk.tile([128, 8], F32, tag="ncv5")
        tv5 = wk.tile([128, 8, 4], F32, tag="tv5")
        for m in range(8):
            nc.vector.tensor_scalar(tv5[:, m, 0:1], statg5[:, m, 0:1], 1.0 / CNT_Y,
                                    None, ALU.mult)
            nc.vector.tensor_scalar(tv5[:, m, 1:2], statg5[:, m, 1:2], 1.0 / CNT_Y,
                                    None, ALU.mult)
            nc.vector.tensor_mul(tv5[:, m, 2:3], tv5[:, m, 0:1], tv5[:, m, 0:1])
            nc.vector.tensor_sub(tv5[:, m, 1:2], tv5[:, m, 1:2], tv5[:, m, 2:3])
            nc.vector.tensor_scalar(tv5[:, m, 1:2], tv5[:, m, 1:2], EPS, None, ALU.add)
            nc.vector.reciprocal(tv5[:, m, 2:3], tv5[:, m, 1:2])
            nc.scalar.activation(tv5[:, m, 3:4], tv5[:, m, 2:3], AF.Sqrt)
            nc.vector.tensor_mul(av5[:, m:m + 1], tv5[:, m, 3:4], g5v[:, m:m + 1])
            nc.vector.tensor_mul(tv5[:, m, 2:3], av5[:, m:m + 1], tv5[:, m, 0:1])
            nc.vector.tensor_sub(cv5[:, m:m + 1], b5v[:, m:m + 1], tv5[:, m, 2:3])
            nc.vector.tensor_scalar(nav5[:, m:m + 1], av5[:, m:m + 1], -1.0, None,
                                    ALU.mult)
            nc.vector.tensor_scalar(ncv5[:, m:m + 1], cv5[:, m:m + 1], -1.0, None,
                                    ALU.mult)

        # apply + pools (recompute y)
        gf = wk.tile([128, 16, 2], F32, tag="gf")  # blocks 0-7 max, 8-15 avg
        pacc = wk.tile([128, 8, 4], F32, tag="pacc")
        qacc = wk.tile([128, 8, 4], F32, tag="qacc")
        mxc = wk.tile([128, 8, 4], F32, tag="mxc")
        for s in range(BL):
            xcb = xc_rounded(s)
            for m in range(8):
                ms = slice(128 * m, 128 * (m + 1))
                for ch in range(2):
                    cs = slice(512 * ch, 512 * (ch + 1))
                    py = ps.tile([128, 512], F32, name=f"pp{s}{m}{ch}", tag="ps")
                    for i in range(5):
                        nc.tensor.matmul(py[:], R(w5_t[0:W5BLK[i][1], i, ms]),
                                         R(xcb[i][:, cs]),
                                         start=(i == 0), stop=(i == 4))
                    idx = s * 2 + ch
                    pr = wk.tile([128, 512], F32, tag="trash", bufs=2)
                    nc.scalar.activation(pr[:], py[:], AF.Relu,
                                         bias=cv5[:, m:m + 1], scale=av5[:, m:m + 1],
                                         accum_out=pacc[:, m, idx:idx + 1])
                    qr = wk.tile([128, 512], F32, tag="trash", bufs=2)
                    nc.scalar.activation(qr[:], py[:], AF.Relu,
                                         bias=ncv5[:, m:m + 1], scale=nav5[:, m:m + 1],
                                         accum_out=qacc[:, m, idx:idx + 1])
                    nc.vector.tensor_reduce(mxc[:, m, idx:idx + 1], py[:],
                                            AX.X, ALU.max)
            # per-sample pooling
            for m in range(8):
                i0, i1 = s * 2, s * 2 + 1
                mx = wk.tile([128, 1], F32, tag="mx5", bufs=2)
                nc.vector.tensor_max(mx[:], mxc[:, m, i0:i0 + 1], mxc[:, m, i1:i1 + 1])
                pm = wk.tile([128, 1], F32, tag="pm5", bufs=2)
                qm = wk.tile([128, 1], F32, tag="qm5", bufs=2)
                nc.scalar.activation(pm[:], mx[:], AF.Relu, bias=cv5[:, m:m + 1],
                                     scale=av5[:, m:m + 1])
                nc.scalar.activation(qm[:], mx[:], AF.Relu, bias=ncv5[:, m:m + 1],
                                     scale=nav5[:, m:m + 1])
                nc.vector.tensor_scalar(qm[:], qm[:], SLOPE, None, ALU.mult)
                nc.vector.tensor_sub(gf[:, m, s:s + 1], pm[:], qm[:])
                t = wk.tile([128, 2], F32, tag="tavg", bufs=2)
                nc.vector.tensor_add(t[:, 0:1], pacc[:, m, i0:i0 + 1],
                                     pacc[:, m, i1:i1 + 1])
                nc.vector.tensor_add(t[:, 1:2], qacc[:, m, i0:i0 + 1],
                                     qacc[:, m, i1:i1 + 1])
                nc.vector.tensor_scalar(t[:, 1:2], t[:, 1:2], SLOPE, None, ALU.mult)
                nc.vector.tensor_sub(t[:, 0:1], t[:, 0:1], t[:, 1:2])
                nc.vector.tensor_scalar(gf[:, 8 + m, s:s + 1], t[:, 0:1], 1.0 / N,
                                        None, ALU.mult)

        # ================= head =================
        def bn_head(h_sb, Mt_, gd, bd, ar_name):
            st = wk.tile([128, Mt_, 2], F32, tag=f"st_{ar_name}")
            for m in range(Mt_):
                nc.vector.tensor_add(st[:, m, 0:1], h_sb[:, m, 0:1], h_sb[:, m, 1:2])
                sq = wk.tile([128, 2], F32, tag=f"sq_{ar_name}", bufs=2)
                nc.scalar.activation(sq[:], h_sb[:, m, :], AF.Square)
                nc.vector.tensor_add(st[:, m, 1:2], sq[:, 0:1], sq[:, 1:2])
            bi = dr.tile([128, Mt_ * 2], F32, name=f"bih_{ar_name}")
            bo = dr.tile([128, Mt_ * 2], F32, name=f"boh_{ar_name}")
            nc.sync.dma_start(bi[:], st[:].rearrange("p a b -> p (a b)"))
            nc.gpsimd.collective_compute("AllReduce", ALU.add,
                                         replica_groups=[list(range(N_CORES))],
                                         ins=[bi.opt()], outs=[bo.opt()])
            sg = wk.tile([128, Mt_, 2], F32, tag=f"sg_{ar_name}")
            nc.sync.dma_start(sg[:].rearrange("p a b -> p (a b)"), bo[:])
            gv = wk.tile([128, Mt_], F32, tag=f"gv_{ar_name}")
            bv = wk.tile([128, Mt_], F32, tag=f"bv_{ar_name}")
            nc.sync.dma_start(gv[:], gd.rearrange("(a p) -> p a", p=128))
            nc.sync.dma_start(bv[:], bd.rearrange("(a p) -> p a", p=128))
            t = wk.tile([128, Mt_, 4], F32, tag=f"t_{ar_name}")
            for m in range(Mt_):
                nc.vector.tensor_scalar(t[:, m, 0:1], sg[:, m, 0:1], 1.0 / CNT_H,
                                        None, ALU.mult)
                nc.vector.tensor_scalar(t[:, m, 1:2], sg[:, m, 1:2], 1.0 / CNT_H,
                                        None, ALU.mult)
                nc.vector.tensor_mul(t[:, m, 2:3], t[:, m, 0:1], t[:, m, 0:1])
                nc.vector.tensor_sub(t[:, m, 1:2], t[:, m, 1:2], t[:, m, 2:3])
                nc.vector.tensor_scalar(t[:, m, 1:2], t[:, m, 1:2], EPS, None, ALU.add)
                nc.vector.reciprocal(t[:, m, 2:3], t[:, m, 1:2])
                nc.scalar.activation(t[:, m, 3:4], t[:, m, 2:3], AF.Sqrt)
                av_ = wk.tile([128, 1], F32, tag=f"av_{ar_name}", bufs=2)
                cv_ = wk.tile([128, 1], F32, tag=f"cv_{ar_name}", bufs=2)
                nc.vector.tensor_mul(av_[:], t[:, m, 3:4], gv[:, m:m + 1])
                nc.vector.tensor_mul(t[:, m, 2:3], av_[:], t[:, m, 0:1])
                nc.vector.tensor_sub(cv_[:], bv[:, m:m + 1], t[:, m, 2:3])
                nc.scalar.activation(h_sb[:, m, :], h_sb[:, m, :], AF.Relu,
                                     bias=cv_[:], scale=av_[:])

        fw1_t = wk.tile([128, 16, 256], F32, tag="bigB")
        for i in range(16):
            nc.sync.dma_start(fw1_t[:, i, :], fw1T_d[128 * i:128 * (i + 1), :])
        h1 = wk.tile([128, 2, 2], F32, tag="h1h")
        for m in range(2):
            ph = ps.tile([128, 2], F32, name=f"ph1{m}", tag="ps")
            for i in range(16):
                nc.tensor.matmul(ph[:], fw1_t[:, i, 128 * m:128 * (m + 1)],
                                 gf[:, i, :], start=(i == 0), stop=False)
            nc.tensor.matmul(ph[:], fb1row[:, 128 * m:128 * (m + 1)], ones2[:],
                             start=False, stop=True)
            nc.scalar.activation(h1[:, m, :], ph[:], AF.Copy)
        bn_head(h1, 2, fg1_d, fbb1_d, "h1")
        h2 = wk.tile([128, 2, 2], F32, tag="h2h")
        for m in range(2):
            ph = ps.tile([128, 2], F32, name=f"ph2{m}", tag="ps")
            for i in range(2):
                nc.tensor.matmul(ph[:], fw2_sb[i][:, 128 * m:128 * (m + 1)],
                                 h1[:, i, :], start=(i == 0), stop=False)
            nc.tensor.matmul(ph[:], fb2row[:, 128 * m:128 * (m + 1)], ones2[:],
                             start=False, stop=True)
            nc.scalar.activation(h2[:, m, :], ph[:], AF.Copy)
        bn_head(h2, 2, fg2_d, fbb2_d, "h2")
        lg = wk.tile([2, CLS], F32, tag="lg")
        for ch, (c0, cw) in enumerate([(0, 512), (512, CLS - 512)]):
            pl = ps.tile([2, 512], F32, name=f"pl{ch}", tag="ps")
            for i in range(2):
                nc.tensor.matmul(pl[:, 0:cw], h2[:, i, :], fw3_sb[i][:, c0:c0 + cw],
                                 start=(i == 0), stop=False)
            nc.tensor.matmul(pl[:, 0:cw], ones2[:], fb3row[:, c0:c0 + cw],
                             start=False, stop=True)
            nc.scalar.activation(lg[:, c0:c0 + cw], pl[:, 0:cw], AF.Copy)
        mxl = wk.tile([2, 4], F32, tag="mxl")
        nc.vector.tensor_reduce(mxl[:, 0:1], lg[:], AX.X, ALU.max)
        nc.vector.tensor_scalar(mxl[:, 1:2], mxl[:, 0:1], -1.0, None, ALU.mult)
        ex = wk.tile([2, CLS], F32, tag="exh")
        nc.scalar.activation(ex[:], lg[:], AF.Exp, bias=mxl[:, 1:2],
                             accum_out=mxl[:, 2:3])
        nc.scalar.activation(mxl[:, 3:4], mxl[:, 2:3], AF.Ln)
        nc.vector.tensor_add(mxl[:, 3:4], mxl[:, 3:4], mxl[:, 0:1])
        nc.vector.tensor_scalar(mxl[:, 3:4], mxl[:, 3:4], -1.0, None, ALU.mult)
        outt = wk.tile([2, CLS], F32, tag="outh")
        nc.scalar.activation(outt[:], lg[:], AF.Identity, bias=mxl[:, 3:4])
        nc.sync.dma_start(out_d[:], outt[:])

    nc.compile()
    return nc


_NC_CACHE = None


def build_in_maps(inputs):
    pts = np.asarray(inputs["pts"], dtype=np.float32)

    def T(x):
        return np.ascontiguousarray(np.asarray(x, dtype=np.float32).T)

    base = {}
    for li in range(4):
        C = LAYERS[li][0]
        w = np.asarray(inputs[f"w{li + 1}"], dtype=np.float32)
        base[f"wn{li}"] = np.ascontiguousarray(w[:, :C].T)
        base[f"wd{li}"] = np.ascontiguousarray((w[:, C:] - w[:, :C]).T)
        base[f"g{li}"] = np.asarray(inputs[f"g{li + 1}"], dtype=np.float32)
        base[f"b{li}"] = np.asarray(inputs[f"b{li + 1}"], dtype=np.float32)
    base["w5T"] = T(inputs["w5"])
    base["g5"] = np.asarray(inputs["g5"], dtype=np.float32)
    base["b5"] = np.asarray(inputs["b5"], dtype=np.float32)
    base["fw1T"] = np.vstack([T(inputs["fw1"]),
                              np.asarray(inputs["fb1"], np.float32)[None, :]])
    base["fg1"] = np.asarray(inputs["fg1"], np.float32)
    base["fbb1"] = np.asarray(inputs["fbb1"], np.float32)
    base["fw2T"] = np.vstack([T(inputs["fw2"]),
                              np.asarray(inputs["fb2"], np.float32)[None, :]])
    base["fg2"] = np.asarray(inputs["fg2"], np.float32)
    base["fbb2"] = np.asarray(inputs["fbb2"], np.float32)
    base["fw3T"] = np.vstack([T(inputs["fw3"]),
                              np.asarray(inputs["fb3"], np.float32)[None, :]])

    in_maps = []
    for c in range(N_CORES):
        m = dict(base)
        shard = pts[c * BL:(c + 1) * BL]
        m["ptsT"] = np.ascontiguousarray(shard.transpose(0, 2, 1))
        in_maps.append(m)
    return in_maps


def kernel(**inputs):
    global _NC_CACHE
    if _NC_CACHE is None:
        _NC_CACHE = build()
    nc = _NC_CACHE
    in_maps = build_in_maps(inputs)
    res = bass_utils.run_bass_kernel_spmd(nc, in_maps, core_ids=list(range(N_CORES)))
    out = np.concatenate([res.results[c]["out"] for c in range(N_CORES)], axis=0)
    return out.astype(np.float32)


# revision 22
# speedup vs baseline: 1.5868x; 1.0654x over previous
"""DGCNN-ReID forward pass on 8 Trainium2 NeuronCores (Bass/Tile).

Data-parallel over batch (16 samples -> 2 per core). Per EdgeConv layer:
the kNN affinity matrix is built on the TensorEngine, exact top-20
neighbor indices are extracted with DVE max8/max_index/match_replace
rounds (stable-argsort semantics, tie-safe), and neighbor features are
fetched with GPSIMD indirect_copy gathers from a feature-major u table,
followed by a max-reduce over the 20 neighbors. Training-mode BatchNorm
statistics: layers 1-2 accumulate E[y], E[y^2] directly from the gathered
edge values (avoids catastrophic cancellation of the sum-decomposition at
small feature scales); layers 3-4 use one cumulative top-k mask matmul
per sample (bf16) for the neighbor sums. Stats are AllReduced across the
8 cores.
"""
import sys
sys.path.insert(0, '/opt/trn_rl_repo')
import numpy as np
from contextlib import ExitStack

import concourse.bass as bass
import concourse.tile as tile
from concourse import bacc, mybir
from concourse import bass_utils

dt = mybir.dt
F32 = dt.float32
F32R = dt.float32r
BF16 = dt.bfloat16
U16 = dt.uint16
AF = mybir.ActivationFunctionType
ALU = mybir.AluOpType
AX = mybir.AxisListType

N_CORES = 8
B, BL, N, K = 16, 2, 1024, 20
NQ = N // 128
EPS = 1e-5
SLOPE = 0.2
EMB = 1024
CLS = 751
LAYERS = [(3, 64), (64, 64), (64, 128), (128, 256)]
DIRECT = [True, True, False, False]
CNT_EDGE = float(B * N * K)
CNT_Y = float(B * N)
CNT_H = float(B)
NEG = -1e30


def R(ap):
    return ap.bitcast(F32R)


def kb_blocks(C):
    out = []
    c = 0
    while c < C:
        out.append((c, min(128, C - c)))
        c += 128
    return out


def build():
    nc = bacc.Bacc("TRN2", target_bir_lowering=False, debug=False,
                   enable_asserts=True, num_devices=N_CORES)

    ptsT = nc.dram_tensor("ptsT", [BL, 3, N], F32, kind="ExternalInput").ap()
    wn_d, wd_d, gm_d, bt_d = [], [], [], []
    for li, (C, O) in enumerate(LAYERS):
        wn_d.append(nc.dram_tensor(f"wn{li}", [C, O], F32, kind="ExternalInput").ap())
        wd_d.append(nc.dram_tensor(f"wd{li}", [C, O], F32, kind="ExternalInput").ap())
        gm_d.append(nc.dram_tensor(f"g{li}", [O], F32, kind="ExternalInput").ap())
        bt_d.append(nc.dram_tensor(f"b{li}", [O], F32, kind="ExternalInput").ap())
    w5T_d = nc.dram_tensor("w5T", [512, EMB], F32, kind="ExternalInput").ap()
    g5_d = nc.dram_tensor("g5", [EMB], F32, kind="ExternalInput").ap()
    b5_d = nc.dram_tensor("b5", [EMB], F32, kind="ExternalInput").ap()
    fw1T_d = nc.dram_tensor("fw1T", [2 * EMB + 1, 256], F32, kind="ExternalInput").ap()
    fg1_d = nc.dram_tensor("fg1", [256], F32, kind="ExternalInput").ap()
    fbb1_d = nc.dram_tensor("fbb1", [256], F32, kind="ExternalInput").ap()
    fw2T_d = nc.dram_tensor("fw2T", [257, 256], F32, kind="ExternalInput").ap()
    fg2_d = nc.dram_tensor("fg2", [256], F32, kind="ExternalInput").ap()
    fbb2_d = nc.dram_tensor("fbb2", [256], F32, kind="ExternalInput").ap()
    fw3T_d = nc.dram_tensor("fw3T", [257, CLS], F32, kind="ExternalInput").ap()
    out_d = nc.dram_tensor("out", [BL, CLS], F32, kind="ExternalOutput").ap()

    with tile.TileContext(nc) as tc, ExitStack() as ctx:
        sb = ctx.enter_context(tc.tile_pool(name="sb", bufs=1))
        wk = ctx.enter_context(tc.tile_pool(name="wk", bufs=1))
        ps = ctx.enter_context(tc.tile_pool(name="ps", bufs=8, space="PSUM"))
        dr = ctx.enter_context(tc.tile_pool(name="dr", bufs=1, space="DRAM"))

        ones_row = sb.tile([1, N], F32)
        nc.vector.memset(ones_row[:], 1.0)
        ones2 = sb.tile([1, 2], F32)
        nc.vector.memset(ones2[:], 1.0)
        iot = sb.tile([128, 128], F32)
        nc.gpsimd.iota(iot[:], pattern=[[1, 128]], base=0, channel_multiplier=-1,
                       allow_small_or_imprecise_dtypes=True)
        identb = sb.tile([128, 128], BF16)
        nc.vector.tensor_scalar(identb[:], iot[:], 0.0, None, ALU.is_equal)

        # ---- load weights ----
        wn_sb, wd_sb = [], []
        for li, (C, O) in enumerate(LAYERS):
            t1 = sb.tile([C, O], F32, name=f"wn{li}")
            nc.sync.dma_start(t1[:], wn_d[li][:, :])
            wn_sb.append(t1)
            t2 = sb.tile([C, O], F32, name=f"wd{li}")
            nc.sync.dma_start(t2[:], wd_d[li][:, :])
            wd_sb.append(t2)
        W5BLK = [(0, 64), (64, 64), (128, 128), (256, 128), (384, 128)]
        fb1row = sb.tile([1, 256], F32)
        nc.sync.dma_start(fb1row[:], fw1T_d[2 * EMB:2 * EMB + 1, :])
        fw2_sb = []
        for i, (c0, cb) in enumerate(kb_blocks(256)):
            t = sb.tile([cb, 256], F32, name=f"fw2_{i}")
            nc.sync.dma_start(t[:], fw2T_d[c0:c0 + cb, :])
            fw2_sb.append(t)
        fb2row = sb.tile([1, 256], F32)
        nc.sync.dma_start(fb2row[:], fw2T_d[256:257, :])
        fw3_sb = []
        for i, (c0, cb) in enumerate(kb_blocks(256)):
            t = sb.tile([cb, CLS], F32, name=f"fw3_{i}")
            nc.sync.dma_start(t[:], fw3T_d[c0:c0 + cb, :])
            fw3_sb.append(t)
        fb3row = sb.tile([1, CLS], F32)
        nc.sync.dma_start(fb3row[:], fw3T_d[256:257, :])

        # persistent x_l.T feature tiles (layer 0 input = pts)
        xs = [[sb.tile([LAYERS[li][0], N], F32, name=f"x{li}_{s}")
               for s in range(BL)] for li in range(4)]
        x4 = [[sb.tile([128, N], F32, name=f"x4_{s}_{i}") for i in range(2)]
              for s in range(BL)]
        for s in range(BL):
            nc.sync.dma_start(xs[0][s][:], ptsT[s])

        def xnext_tiles(li, s):
            # output z.T destination tiles for layer li: list of (tile, ob)
            if li + 1 < 4:
                return [(xs[li + 1][s], LAYERS[li][1])]
            return [(x4[s][0], 128), (x4[s][1], 128)]

        # ================= edge layers =================
        for li, (C, O) in enumerate(LAYERS):
            Mt = (O + 127) // 128
            direct = DIRECT[li]
            if direct:
                syp = wk.tile([128, BL * 4, NQ], F32, tag="syp")
                sy2p = wk.tile([128, BL * 4], F32, tag="sy2p")
            else:
                sup = wk.tile([128, Mt, BL * NQ], F32, tag="syp")
                sqp = wk.tile([128, Mt, BL * NQ], F32, tag="sy2p")
                crp = wk.tile([128, Mt, BL * NQ], F32, tag="crp")
                scp = wk.tile([128, Mt, BL * 2], F32, tag="scp")
                scqp = wk.tile([128, Mt, BL * 2], F32, tag="scqp")

            st_ = [dict() for _ in range(BL)]

            def prep(s):
                d = st_[s]
                xb = xs[li][s]
                xsq = wk.tile([C, N], F32, tag="xsq")
                nc.scalar.activation(xsq[:], xb[:], AF.Square)
                onesc = wk.tile([C, 1], F32, tag="onesc", bufs=2)
                nc.vector.memset(onesc[:], 1.0)
                xxneg = wk.tile([1, N], F32, tag="xxneg")
                for ch in range(2):
                    cs = slice(512 * ch, 512 * (ch + 1))
                    pxx = ps.tile([1, 512], F32, name=f"pxx{li}{s}{ch}", tag="ps")
                    nc.tensor.matmul(pxx[:], onesc[:], xsq[:, cs], start=True,
                                     stop=True)
                    nc.scalar.activation(xxneg[:, cs], pxx[:], AF.Identity,
                                         scale=-0.5)
                d["xxneg"] = xxneg
                uTs = []
                for m in range(Mt):
                    ob = min(128, O - 128 * m)
                    ut = wk.tile([128, N], F32, tag="uT0", bufs=2)
                    for ch in range(2):
                        cs = slice(512 * ch, 512 * (ch + 1))
                        pu = ps.tile([ob, 512], F32, name=f"pu{li}{s}{m}{ch}",
                                     tag="ps")
                        nc.tensor.matmul(pu[:], wn_sb[li][:, 128 * m:128 * m + ob],
                                         xb[:, cs], start=True, stop=True)
                        nc.scalar.activation(ut[0:ob, cs], pu[:], AF.Copy)
                    uTs.append(ut)
                if O == 64:
                    nc.sync.dma_start(uTs[0][64:128, :], uTs[0][0:64, :])
                d["uTs"] = uTs
                cT = wk.tile([128, Mt, N], F32, tag="cT")
                for m in range(Mt):
                    ob = min(128, O - 128 * m)
                    for ch in range(2):
                        cs = slice(512 * ch, 512 * (ch + 1))
                        pc = ps.tile([ob, 512], F32, name=f"pc{li}{s}{m}{ch}",
                                     tag="ps")
                        nc.tensor.matmul(pc[:], wd_sb[li][:, 128 * m:128 * m + ob],
                                         xb[:, cs], start=True, stop=True)
                        if direct:
                            nc.scalar.activation(cT[0:ob, m, cs], pc[:], AF.Copy)
                        else:
                            nc.scalar.activation(cT[0:ob, m, cs], pc[:],
                                                 AF.Identity,
                                                 accum_out=scp[0:ob, m,
                                                              2 * s + ch:2 * s + ch + 1])
                            tr = wk.tile([128, 512], F32, tag="trash")
                            nc.scalar.activation(tr[0:ob, :], cT[0:ob, m, cs],
                                                 AF.Square,
                                                 accum_out=scqp[0:ob, m,
                                                               2 * s + ch:2 * s + ch + 1])
                d["cT"] = cT
                if direct:
                    # rows 0-63: c.T; rows 64-127: c.T shifted left 128 cols
                    cd = wk.tile([128, N], F32, tag="cdup", bufs=2)
                    nc.sync.dma_start(cd[0:64, :], cT[0:64, 0, :])
                    nc.sync.dma_start(cd[64:128, 0:N - 128], cT[0:64, 0, 128:N])
                    d["cd"] = cd
                else:
                    upm = wk.tile([128, NQ, O], BF16, tag="upm")
                    usq = wk.tile([128, NQ, O], BF16, tag="usq")
                    for ib in range(NQ):
                        pp = ps.tile([128, O], F32, name=f"pp{li}{s}{ib}", tag="ps")
                        nc.tensor.matmul(pp[:], xb[:, 128 * ib:128 * (ib + 1)],
                                         wn_sb[li][:, :], start=True, stop=True)
                        nc.scalar.activation(upm[:, ib, :], pp[:], AF.Copy)
                        nc.scalar.activation(usq[:, ib, :], upm[:, ib, :], AF.Square)
                    d["upm"], d["usq"] = upm, usq

            def smm_phase(s):
                # PE + scalar only: fill s_q tiles for all 8 dest blocks
                d = st_[s]
                xb = xs[li][s]
                xxneg = d["xxneg"]
                sqs = []
                for q in range(NQ):
                    qs = slice(128 * q, 128 * (q + 1))
                    sq_ = wk.tile([128, N], F32, tag="sq", bufs=3)
                    for ch in range(2):
                        cs = slice(512 * ch, 512 * (ch + 1))
                        pss = ps.tile([128, 512], F32, name=f"pss{li}{s}{q}{ch}",
                                      tag="ps")
                        nc.tensor.matmul(pss[:], xb[:, qs], xb[:, cs],
                                         start=True, stop=False)
                        nc.tensor.matmul(pss[:], ones_row[:, qs], xxneg[:, cs],
                                         start=False, stop=True)
                        nc.scalar.activation(sq_[:, cs], pss[:], AF.Copy)
                    sqs.append(sq_)
                d["sqs"] = sqs

            def dve_phase(s):
                # DVE top-20 chains; mask path + per-q stats for formula layers
                d = st_[s]
                cT = d.get("cT")
                i24 = wk.tile([128, NQ, 24], U16, tag="i24", bufs=2)
                pend = []

                def flush_pend():
                    for (ob_, pS_, m_, ci_, qs_) in pend:
                        trd = wk.tile([128, 512], F32, tag="trashd")
                        nc.vector.scalar_tensor_tensor(
                            trd[0:ob_, 0:128], pS_[0:ob_, :], 1.0,
                            cT[0:ob_, m_, qs_], ALU.mult, ALU.mult,
                            accum_out=crp[0:ob_, m_, ci_:ci_ + 1])
                    pend.clear()

                for q in range(NQ):
                    qs = slice(128 * q, 128 * (q + 1))
                    sq_ = d["sqs"][q]
                    v24 = wk.tile([128, 24], F32, tag="v24", bufs=2)
                    nc.vector.max(v24[:, 0:8], sq_[:])
                    nc.vector.max_index(i24[:, q, 0:8], v24[:, 0:8], sq_[:])
                    sq2 = wk.tile([128, N], F32, tag="sqr", bufs=2)
                    nc.vector.match_replace(sq2[:], v24[:, 0:8], sq_[:], NEG)
                    nc.vector.max(v24[:, 8:16], sq2[:])
                    nc.vector.max_index(i24[:, q, 8:16], v24[:, 8:16], sq2[:])
                    sq3 = wk.tile([128, N], F32, tag="sqr", bufs=2)
                    nc.vector.match_replace(sq3[:], v24[:, 8:16], sq2[:], NEG)
                    nc.vector.max(v24[:, 16:24], sq3[:])
                    nc.vector.max_index(i24[:, q, 16:24], v24[:, 16:24], sq3[:])
                    if not direct:
                        mq = wk.tile([128, N], BF16, tag="mskq", bufs=2)
                        nc.vector.tensor_scalar(mq[:], sq_[:], v24[:, 19:20],
                                                None, ALU.is_ge)
                        # lag-1 cross-term so DVE never waits on PE here
                        flush_pend()
                        mskq_t = wk.tile([128, NQ, 128], BF16, tag="mskT", bufs=2)
                        for jb in range(NQ):
                            pt = ps.tile([128, 128], BF16,
                                         name=f"pt{li}{s}{q}{jb}", tag="ps")
                            nc.tensor.transpose(pt[:], mq[:, 128 * jb:128 * (jb + 1)],
                                                identb[:])
                            nc.scalar.activation(mskq_t[:, jb, :], pt[:], AF.Copy)
                        upm, usq = d["upm"], d["usq"]
                        for m in range(Mt):
                            ob = min(128, O - 128 * m)
                            ms = slice(128 * m, 128 * m + ob)
                            pS = ps.tile([128, 128], F32,
                                         name=f"pS{li}{s}{q}{m}", tag="ps")
                            pQ = ps.tile([128, 128], F32,
                                         name=f"pQ{li}{s}{q}{m}", tag="ps")
                            for jb in range(NQ):
                                nc.tensor.matmul(pS[0:ob, :], upm[:, jb, ms],
                                                 mskq_t[:, jb, :],
                                                 start=(jb == 0), stop=(jb == NQ - 1))
                                nc.tensor.matmul(pQ[0:ob, :], usq[:, jb, ms],
                                                 mskq_t[:, jb, :],
                                                 start=(jb == 0), stop=(jb == NQ - 1))
                            ci = s * NQ + q
                            tr = wk.tile([128, 512], F32, tag="trash")
                            nc.scalar.activation(tr[0:ob, 0:128], pS[0:ob, :],
                                                 AF.Identity,
                                                 accum_out=sup[0:ob, m, ci:ci + 1])
                            tr2 = wk.tile([128, 512], F32, tag="trash")
                            nc.scalar.activation(tr2[0:ob, 0:128], pQ[0:ob, :],
                                                 AF.Identity,
                                                 accum_out=sqp[0:ob, m, ci:ci + 1])
                            pend.append((ob, pS, m, ci, qs))
                if not direct:
                    flush_pend()
                d["i24"] = i24

            def idx_dma(s):
                # list order t = 16*(j*20 + k) + p for dest i = 16*j + p ->
                # contiguous 20/160-element u16 runs; G is [ch, j, k, p(16)]
                d = st_[s]
                i24 = d["i24"]
                dram_w = dr.tile([NQ, 16, 160], U16, name=f"idxd{li}{s}")
                for j in range(NQ):
                    d1 = bass.AP(dram_w.tensor, dram_w[:].offset + 20 * j,
                                 [[160, 16], [2560, NQ], [1, 20]])
                    nc.sync.dma_start(d1, i24[16 * j:16 * (j + 1), :, 0:20])
                if O == 64:
                    idxw = wk.tile([128, 4, 192], U16, tag="idxw", bufs=2)
                    for g in range(8):
                        srcg = bass.AP(dram_w.tensor,
                                       dram_w[:].offset + 2560 * (g // 4),
                                       [[160, 16], [5120, 4], [1, 160]])
                        dslc = idxw[16 * g:16 * (g + 1), :, :]
                        dstg = bass.AP(idxw.tensor, dslc.offset,
                                       [dslc.ap[0], [192, 4], [1, 160]])
                        nc.sync.dma_start(dstg, srcg)
                else:
                    idxw = wk.tile([128, NQ, 192], U16, tag="idxw", bufs=2)
                    for g in range(8):
                        srcg = bass.AP(dram_w.tensor, dram_w[:].offset,
                                       [[160, 16], [2560, NQ], [1, 160]])
                        dslc = idxw[16 * g:16 * (g + 1), :, :]
                        dstg = bass.AP(idxw.tensor, dslc.offset,
                                       [dslc.ap[0], [192, NQ], [1, 160]])
                        nc.sync.dma_start(dstg, srcg)
                d["idxw"] = idxw

            def kmax(G, ob):
                # max over k: G is [128, j(8), k(20), p(16)]; transposed AP
                # view puts k innermost; out columns are dest-ordered.
                Gv = G[:].rearrange("p (j k i) -> p j i k", k=20, i=16)
                zt = wk.tile([128, 128], F32, tag="zt", bufs=2)
                nc.vector.tensor_reduce(
                    zt[0:ob, :].rearrange("p (j i) -> p j i", i=16),
                    Gv[0:ob], AX.X, ALU.max)
                return zt

            def gather_phase(s):
                d = st_[s]
                idxw = d["idxw"]
                uTs, cT = d["uTs"], d["cT"]
                if direct:
                    cd = d["cd"]
                    for t in range(4):
                        G = wk.tile([128, 2560], F32, tag="G", bufs=3)
                        for a, b in [(0, 64), (64, 128), (128, 160)]:
                            nc.gpsimd.indirect_copy(
                                G[:, 16 * a:16 * b].rearrange("p (i o) -> p i o", o=1),
                                uTs[0][:], idxw[:, t, a:b], True)
                        cds = cd[:, 256 * t:256 * t + 128]
                        tG = wk.tile([128, 2560], F32, tag="G", bufs=3)
                        tGv = tG[:].rearrange("p (j k i) -> p j k i", k=20, i=16)
                        Gvv = G[:].rearrange("p (j k i) -> p j k i", k=20, i=16)
                        for j in range(NQ):
                            cb = bass.AP(cd.tensor, cds.offset + 16 * j,
                                         [cds.ap[0], [0, 20], [1, 16]])
                            nc.vector.scalar_tensor_tensor(
                                tGv[:, j], Gvv[:, j], 0.0, cb, ALU.add, ALU.add,
                                accum_out=syp[:, s * 4 + t, j:j + 1])
                        nc.scalar.activation(tG[:], tG[:], AF.Square,
                                             accum_out=sy2p[:, s * 4 + t:s * 4 + t + 1])
                        zt = kmax(G, 128)
                        dst, _ = xnext_tiles(li, s)[0]
                        nc.vector.tensor_add(dst[0:64, 256 * t:256 * t + 128],
                                             zt[0:64, :],
                                             cd[0:64, 256 * t:256 * t + 128])
                        zhi = wk.tile([128, 128], F32, tag="zt", bufs=2)
                        nc.vector.tensor_add(zhi[64:128, :], zt[64:128, :],
                                             cd[64:128, 256 * t:256 * t + 128])
                        nc.sync.dma_start(dst[0:64, 256 * t + 128:256 * t + 256],
                                          zhi[64:128, :])
                else:
                    for q in range(NQ):
                        for m in range(Mt):
                            ob = min(128, O - 128 * m)
                            G = wk.tile([128, 2560], F32, tag="G", bufs=3)
                            for a, b in [(0, 64), (64, 128), (128, 160)]:
                                nc.gpsimd.indirect_copy(
                                    G[:, 16 * a:16 * b].rearrange("p (i o) -> p i o", o=1),
                                    uTs[m][:], idxw[:, q, a:b], True)
                            zt = kmax(G, ob)
                            dst, _ = xnext_tiles(li, s)[m]
                            nc.vector.tensor_add(dst[0:ob, 128 * q:128 * (q + 1)],
                                                 zt[0:ob, :],
                                                 cT[0:ob, m, 128 * q:128 * (q + 1)])

            if direct:
                # samples interleaved: DVE goes straight from s0 top-k into
                # s1 top-k while s0's gathers land on GpSimd
                for s in range(BL):
                    prep(s)
                    smm_phase(s)
                    dve_phase(s)
                    idx_dma(s)
                for s in range(BL):
                    gather_phase(s)
            else:
                # cT/uT buffer lifetimes force sample-serial phases here
                for s in range(BL):
                    prep(s)
                    smm_phase(s)
                    dve_phase(s)
                    idx_dma(s)
                    gather_phase(s)

            # ---- finalize stats ----
            stat = wk.tile([128, Mt, 2], F32, tag="stat")
            nc.vector.memset(stat[:].rearrange("p a b -> p (a b)"), 0.0)
            if direct:
                sypf = syp[:].rearrange("p a b -> p (a b)")
                fold1 = wk.tile([64, BL * 4 * NQ], F32, tag="fold1")
                nc.sync.dma_start(fold1[:], sypf[64:128, :])
                fold2 = wk.tile([64, BL * 4], F32, tag="fold2")
                nc.sync.dma_start(fold2[:], sy2p[64:128, :])
                nc.vector.tensor_add(sypf[0:64, :], sypf[0:64, :], fold1[:])
                nc.vector.tensor_add(sy2p[0:64, :], sy2p[0:64, :], fold2[:])
                nc.vector.tensor_reduce(stat[0:64, 0, 0:1], sypf[0:64, :], AX.X,
                                        ALU.add)
                nc.vector.tensor_reduce(stat[0:64, 0, 1:2], sy2p[0:64, :], AX.X,
                                        ALU.add)
            else:
                red = wk.tile([128, Mt, 5], F32, tag="red")
                for m in range(Mt):
                    ob = min(128, O - 128 * m)
                    nc.vector.tensor_reduce(red[0:ob, m, 0:1], sup[0:ob, m, :],
                                            AX.X, ALU.add)
                    nc.vector.tensor_reduce(red[0:ob, m, 1:2], sqp[0:ob, m, :],
                                            AX.X, ALU.add)
                    nc.vector.tensor_reduce(red[0:ob, m, 2:3], crp[0:ob, m, :],
                                            AX.X, ALU.add)
                    nc.vector.tensor_reduce(red[0:ob, m, 3:4], scp[0:ob, m, :],
                                            AX.X, ALU.add)
                    nc.vector.tensor_reduce(red[0:ob, m, 4:5], scqp[0:ob, m, :],
                                            AX.X, ALU.add)
                    # Sy = Su + K*Sc
                    nc.vector.tensor_scalar(stat[0:ob, m, 0:1], red[0:ob, m, 3:4],
                                            float(K), None, ALU.mult)
                    nc.vector.tensor_add(stat[0:ob, m, 0:1], stat[0:ob, m, 0:1],
                                         red[0:ob, m, 0:1])
                    # Sy2 = Sq + 2*cross + K*Scq
                    nc.vector.tensor_scalar(stat[0:ob, m, 1:2], red[0:ob, m, 2:3],
                                            2.0, None, ALU.mult)
                    nc.vector.tensor_add(stat[0:ob, m, 1:2], stat[0:ob, m, 1:2],
                                         red[0:ob, m, 1:2])
                    tk = wk.tile([128, 1], F32, tag="tk", bufs=2)
                    nc.vector.tensor_scalar(tk[0:ob, :], red[0:ob, m, 4:5],
                                            float(K), None, ALU.mult)
                    nc.vector.tensor_add(stat[0:ob, m, 1:2], stat[0:ob, m, 1:2],
                                         tk[0:ob, :])

            # ---- AllReduce + BN coefs + apply ----
            bin_ = dr.tile([128, Mt * 2], F32, name=f"bi{li}")
            bout = dr.tile([128, Mt * 2], F32, name=f"bo{li}")
            nc.sync.dma_start(bin_[:], stat[:].rearrange("p a b -> p (a b)"))
            nc.gpsimd.collective_compute("AllReduce", ALU.add,
                                         replica_groups=[list(range(N_CORES))],
                                         ins=[bin_.opt()], outs=[bout.opt()])
            statg = wk.tile([128, Mt, 2], F32, tag="statg")
            nc.sync.dma_start(statg[:].rearrange("p a b -> p (a b)"), bout[:])
            gamv = wk.tile([128, Mt], F32, tag="gamv")
            betv = wk.tile([128, Mt], F32, tag="betv")
            for m in range(Mt):
                ob = min(128, O - 128 * m)
                nc.sync.dma_start(gamv[0:ob, m:m + 1],
                                  gm_d[li][128 * m:128 * m + ob].rearrange("(p a) -> p a", a=1))
                nc.sync.dma_start(betv[0:ob, m:m + 1],
                                  bt_d[li][128 * m:128 * m + ob].rearrange("(p a) -> p a", a=1))
            av = wk.tile([128, Mt], F32, tag="av")
            cv = wk.tile([128, Mt], F32, tag="cv")
            nav = wk.tile([128, Mt], F32, tag="nav")
            ncv = wk.tile([128, Mt], F32, tag="ncv")
            tv = wk.tile([128, Mt, 4], F32, tag="tv")
            for m in range(Mt):
                ob = min(128, O - 128 * m)
                nc.vector.tensor_scalar(tv[0:ob, m, 0:1], statg[0:ob, m, 0:1],
                                        1.0 / CNT_EDGE, None, ALU.mult)
                nc.vector.tensor_scalar(tv[0:ob, m, 1:2], statg[0:ob, m, 1:2],
                                        1.0 / CNT_EDGE, None, ALU.mult)
                nc.vector.tensor_mul(tv[0:ob, m, 2:3], tv[0:ob, m, 0:1],
                                     tv[0:ob, m, 0:1])
                nc.vector.tensor_sub(tv[0:ob, m, 1:2], tv[0:ob, m, 1:2],
                                     tv[0:ob, m, 2:3])
                nc.vector.tensor_scalar(tv[0:ob, m, 1:2], tv[0:ob, m, 1:2], EPS,
                                        None, ALU.add)
                nc.vector.reciprocal(tv[0:ob, m, 2:3], tv[0:ob, m, 1:2])
                nc.scalar.activation(tv[0:ob, m, 3:4], tv[0:ob, m, 2:3], AF.Sqrt)
                nc.vector.tensor_mul(av[0:ob, m:m + 1], tv[0:ob, m, 3:4],
                                     gamv[0:ob, m:m + 1])
                nc.vector.tensor_mul(tv[0:ob, m, 2:3], av[0:ob, m:m + 1],
                                     tv[0:ob, m, 0:1])
                nc.vector.tensor_sub(cv[0:ob, m:m + 1], betv[0:ob, m:m + 1],
                                     tv[0:ob, m, 2:3])
                nc.vector.tensor_scalar(nav[0:ob, m:m + 1], av[0:ob, m:m + 1], -1.0,
                                        None, ALU.mult)
                nc.vector.tensor_scalar(ncv[0:ob, m:m + 1], cv[0:ob, m:m + 1], -1.0,
                                        None, ALU.mult)
            for s in range(BL):
                for m in range(Mt):
                    ob = min(128, O - 128 * m)
                    dst, _ = xnext_tiles(li, s)[m]
                    p_s = wk.tile([128, N], F32, tag="sq", bufs=3)
                    q_s = wk.tile([128, N], F32, tag="sq", bufs=3)
                    nc.scalar.activation(p_s[0:ob, :], dst[0:ob, :], AF.Relu,
                                         bias=cv[0:ob, m:m + 1],
                                         scale=av[0:ob, m:m + 1])
                    nc.scalar.activation(q_s[0:ob, :], dst[0:ob, :], AF.Relu,
                                         bias=ncv[0:ob, m:m + 1],
                                         scale=nav[0:ob, m:m + 1])
                    nc.vector.tensor_scalar(q_s[0:ob, :], q_s[0:ob, :], SLOPE,
                                            None, ALU.mult)
                    nc.vector.tensor_sub(dst[0:ob, :], p_s[0:ob, :], q_s[0:ob, :])

        # ================= conv5 + pooling =================
        w5_t = wk.tile([128, 5, EMB], F32R, tag="bigB", name="w5_t")
        for i, (c0, cb) in enumerate(W5BLK):
            w5_s = wk.tile([128, EMB], F32, tag="xsq")
            nc.sync.dma_start(w5_s[0:cb, :], w5T_d[c0:c0 + cb, :])
            nc.scalar.activation(w5_t[0:cb, i, :], w5_s[0:cb, :], AF.Copy)

        def xc_blocks(s):
            return [xs[1][s], xs[2][s], xs[3][s], x4[s][0], x4[s][1]]

        XCTAGS = [("cdup", 2), ("xsq", 1), ("uT0", 2), ("G", 3), ("uT0", 2)]

        def xc_rounded(s):
            out = []
            for i, t in enumerate(xc_blocks(s)):
                cb = t.shape[0]
                tg, bf = XCTAGS[i]
                rt = wk.tile([cb, N], F32R, name=f"xcr{s}_{i}", tag=tg, bufs=bf)
                nc.scalar.activation(rt[:], t[:], AF.Copy)
                out.append(rt)
            return out

        s1 = wk.tile([128, 8, 4], F32, tag="s1c")
        s2 = wk.tile([128, 8, 4], F32, tag="s2c")
        for s in range(BL):
            xcb = xc_rounded(s)
            for m in range(8):
                ms = slice(128 * m, 128 * (m + 1))
                for ch in range(2):
                    cs = slice(512 * ch, 512 * (ch + 1))
                    py = ps.tile([128, 512], F32, name=f"pw{s}{m}{ch}", tag="ps")
                    for i in range(5):
                        nc.tensor.matmul(py[:], R(w5_t[0:W5BLK[i][1], i, ms]),
                                         R(xcb[i][:, cs]),
                                         start=(i == 0), stop=(i == 4))
                    idx = s * 2 + ch
                    t1 = wk.tile([128, 512], F32, tag="trash")
                    nc.scalar.activation(t1[:], py[:], AF.Identity,
                                         accum_out=s1[:, m, idx:idx + 1])
                    t2 = wk.tile([128, 512], F32, tag="trash")
                    nc.scalar.activation(t2[:], py[:], AF.Square,
                                         accum_out=s2[:, m, idx:idx + 1])
        stat5 = wk.tile([128, 8, 2], F32, tag="stat5")
        for m in range(8):
            nc.vector.tensor_reduce(stat5[:, m, 0:1], s1[:, m, :], AX.X, ALU.add)
            nc.vector.tensor_reduce(stat5[:, m, 1:2], s2[:, m, :], AX.X, ALU.add)
        bin5 = dr.tile([128, 16], F32, name="bi5")
        bout5 = dr.tile([128, 16], F32, name="bo5")
        nc.sync.dma_start(bin5[:], stat5[:].rearrange("p a b -> p (a b)"))
        nc.gpsimd.collective_compute("AllReduce", ALU.add,
                                     replica_groups=[list(range(N_CORES))],
                                     ins=[bin5.opt()], outs=[bout5.opt()])
        statg5 = wk.tile([128, 8, 2], F32, tag="statg5")
        nc.sync.dma_start(statg5[:].rearrange("p a b -> p (a b)"), bout5[:])
        g5v = wk.tile([128, 8], F32, tag="g5v")
        b5v = wk.tile([128, 8], F32, tag="b5v")
        nc.sync.dma_start(g5v[:], g5_d.rearrange("(a p) -> p a", p=128))
        nc.sync.dma_start(b5v[:], b5_d.rearrange("(a p) -> p a", p=128))
        av5 = wk.tile([128, 8], F32, tag="av5")
        cv5 = wk.tile([128, 8], F32, tag="cv5")
        nav5 = wk.tile([128, 8], F32, tag="nav5")
        ncv5 = wk.tile([128, 8], F32, tag="ncv5")
        tv5 = wk.tile([128, 8, 4], F32, tag="tv5")
        for m in range(8):
            nc.vector.tensor_scalar(tv5[:, m, 0:1], statg5[:, m, 0:1], 1.0 / CNT_Y,
                                    None, ALU.mult)
            nc.vector.tensor_scalar(tv5[:, m, 1:2], statg5[:, m, 1:2], 1.0 / CNT_Y,
                                    None, ALU.mult)
            nc.vector.tensor_mul(tv5[:, m, 2:3], tv5[:, m, 0:1], tv5[:, m, 0:1])
            nc.vector.tensor_sub(tv5[:, m, 1:2], tv5[:, m, 1:2], tv5[:, m, 2:3])
            nc.vector.tensor_scalar(tv5[:, m, 1:2], tv5[:, m, 1:2], EPS, None, ALU.add)
            nc.vector.reciprocal(tv5[:, m, 2:3], tv5[:, m, 1:2])
            nc.scalar.activation(tv5[:, m, 3:4], tv5[:, m, 2:3], AF.Sqrt)
            nc.vector.tensor_mul(av5[:, m:m + 1], tv5[:, m, 3:4], g5v[:, m:m + 1])
            nc.vector.tensor_mul(tv5[:, m, 2:3], av5[:, m:m + 1], tv5[:, m, 0:1])
            nc.vector.tensor_sub(cv5[:, m:m + 1], b5v[:, m:m + 1], tv5[:, m, 2:3])
            nc.vector.tensor_scalar(nav5[:, m:m + 1], av5[:, m:m + 1], -1.0, None,
                                    ALU.mult)
            nc.vector.tensor_scalar(ncv5[:, m:m + 1], cv5[:, m:m + 1], -1.0, None,
                                    ALU.mult)

        # apply + pools (recompute y)
        gf = wk.tile([128, 16, 2], F32, tag="gf")  # blocks 0-7 max, 8-15 avg
        pacc = wk.tile([128, 8, 4], F32, tag="pacc")
        qacc = wk.tile([128, 8, 4], F32, tag="qacc")
        mxc = wk.tile([128, 8, 4], F32, tag="mxc")
        for s in range(BL):
            xcb = xc_rounded(s)
            for m in range(8):
                ms = slice(128 * m, 128 * (m + 1))
                for ch in range(2):
                    cs = slice(512 * ch, 512 * (ch + 1))
                    py = ps.tile([128, 512], F32, name=f"pp{s}{m}{ch}", tag="ps")
                    for i in range(5):
                        nc.tensor.matmul(py[:], R(w5_t[0:W5BLK[i][1], i, ms]),
                                         R(xcb[i][:, cs]),
                                         start=(i == 0), stop=(i == 4))
                    idx = s * 2 + ch
                    pr = wk.tile([128, 512], F32, tag="trash")
                    nc.scalar.activation(pr[:], py[:], AF.Relu,
                                         bias=cv5[:, m:m + 1], scale=av5[:, m:m + 1],
                                         accum_out=pacc[:, m, idx:idx + 1])
                    qr = wk.tile([128, 512], F32, tag="trash")
                    nc.scalar.activation(qr[:], py[:], AF.Relu,
                                         bias=ncv5[:, m:m + 1], scale=nav5[:, m:m + 1],
                                         accum_out=qacc[:, m, idx:idx + 1])
                    nc.vector.tensor_reduce(mxc[:, m, idx:idx + 1], py[:],
                                            AX.X, ALU.max)
            # per-sample pooling
            for m in range(8):
                i0, i1 = s * 2, s * 2 + 1
                mx = wk.tile([128, 1], F32, tag="mx5", bufs=2)
                nc.vector.tensor_max(mx[:], mxc[:, m, i0:i0 + 1], mxc[:, m, i1:i1 + 1])
                pm = wk.tile([128, 1], F32, tag="pm5", bufs=2)
                qm = wk.tile([128, 1], F32, tag="qm5", bufs=2)
                nc.scalar.activation(pm[:], mx[:], AF.Relu, bias=cv5[:, m:m + 1],
                                     scale=av5[:, m:m + 1])
                nc.scalar.activation(qm[:], mx[:], AF.Relu, bias=ncv5[:, m:m + 1],
                                     scale=nav5[:, m:m + 1])
                nc.vector.tensor_scalar(qm[:], qm[:], SLOPE, None, ALU.mult)
                nc.vector.tensor_sub(gf[:, m, s:s + 1], pm[:], qm[:])
                t = wk.tile([128, 2], F32, tag="tavg", bufs=2)
                nc.vector.tensor_add(t[:, 0:1], pacc[:, m, i0:i0 + 1],
                                     pacc[:, m, i1:i1 + 1])
                nc.vector.tensor_add(t[:, 1:2], qacc[:, m, i0:i0 + 1],
                                     qacc[:, m, i1:i1 + 1])
                nc.vector.tensor_scalar(t[:, 1:2], t[:, 1:2], SLOPE, None, ALU.mult)
                nc.vector.tensor_sub(t[:, 0:1], t[:, 0:1], t[:, 1:2])
                nc.vector.tensor_scalar(gf[:, 8 + m, s:s + 1], t[:, 0:1], 1.0 / N,
                                        None, ALU.mult)

        # ================= head =================
        def bn_head(h_sb, Mt_, gd, bd, ar_name):
            st = wk.tile([128, Mt_, 2], F32, tag=f"st_{ar_name}")
            for m in range(Mt_):
                nc.vector.tensor_add(st[:, m, 0:1], h_sb[:, m, 0:1], h_sb[:, m, 1:2])
                sq = wk.tile([128, 2], F32, tag=f"sq_{ar_name}", bufs=2)
                nc.scalar.activation(sq[:], h_sb[:, m, :], AF.Square)
                nc.vector.tensor_add(st[:, m, 1:2], sq[:, 0:1], sq[:, 1:2])
            bi = dr.tile([128, Mt_ * 2], F32, name=f"bih_{ar_name}")
            bo = dr.tile([128, Mt_ * 2], F32, name=f"boh_{ar_name}")
            nc.sync.dma_start(bi[:], st[:].rearrange("p a b -> p (a b)"))
            nc.gpsimd.collective_compute("AllReduce", ALU.add,
                                         replica_groups=[list(range(N_CORES))],
                                         ins=[bi.opt()], outs=[bo.opt()])
            sg = wk.tile([128, Mt_, 2], F32, tag=f"sg_{ar_name}")
            nc.sync.dma_start(sg[:].rearrange("p a b -> p (a b)"), bo[:])
            gv = wk.tile([128, Mt_], F32, tag=f"gv_{ar_name}")
            bv = wk.tile([128, Mt_], F32, tag=f"bv_{ar_name}")
            nc.sync.dma_start(gv[:], gd.rearrange("(a p) -> p a", p=128))
            nc.sync.dma_start(bv[:], bd.rearrange("(a p) -> p a", p=128))
            t = wk.tile([128, Mt_, 4], F32, tag=f"t_{ar_name}")
            for m in range(Mt_):
                nc.vector.tensor_scalar(t[:, m, 0:1], sg[:, m, 0:1], 1.0 / CNT_H,
                                        None, ALU.mult)
                nc.vector.tensor_scalar(t[:, m, 1:2], sg[:, m, 1:2], 1.0 / CNT_H,
                                        None, ALU.mult)
                nc.vector.tensor_mul(t[:, m, 2:3], t[:, m, 0:1], t[:, m, 0:1])
                nc.vector.tensor_sub(t[:, m, 1:2], t[:, m, 1:2], t[:, m, 2:3])
                nc.vector.tensor_scalar(t[:, m, 1:2], t[:, m, 1:2], EPS, None, ALU.add)
                nc.vector.reciprocal(t[:, m, 2:3], t[:, m, 1:2])
                nc.scalar.activation(t[:, m, 3:4], t[:, m, 2:3], AF.Sqrt)
                av_ = wk.tile([128, 1], F32, tag=f"av_{ar_name}", bufs=2)
                cv_ = wk.tile([128, 1], F32, tag=f"cv_{ar_name}", bufs=2)
                nc.vector.tensor_mul(av_[:], t[:, m, 3:4], gv[:, m:m + 1])
                nc.vector.tensor_mul(t[:, m, 2:3], av_[:], t[:, m, 0:1])
                nc.vector.tensor_sub(cv_[:], bv[:, m:m + 1], t[:, m, 2:3])
                nc.scalar.activation(h_sb[:, m, :], h_sb[:, m, :], AF.Relu,
                                     bias=cv_[:], scale=av_[:])

        fw1_t = wk.tile([128, 16, 256], F32, tag="bigB")
        for i in range(16):
            nc.sync.dma_start(fw1_t[:, i, :], fw1T_d[128 * i:128 * (i + 1), :])
        h1 = wk.tile([128, 2, 2], F32, tag="h1h")
        for m in range(2):
            ph = ps.tile([128, 2], F32, name=f"ph1{m}", tag="ps")
            for i in range(16):
                nc.tensor.matmul(ph[:], fw1_t[:, i, 128 * m:128 * (m + 1)],
                                 gf[:, i, :], start=(i == 0), stop=False)
            nc.tensor.matmul(ph[:], fb1row[:, 128 * m:128 * (m + 1)], ones2[:],
                             start=False, stop=True)
            nc.scalar.activation(h1[:, m, :], ph[:], AF.Copy)
        bn_head(h1, 2, fg1_d, fbb1_d, "h1")
        h2 = wk.tile([128, 2, 2], F32, tag="h2h")
        for m in range(2):
            ph = ps.tile([128, 2], F32, name=f"ph2{m}", tag="ps")
            for i in range(2):
                nc.tensor.matmul(ph[:], fw2_sb[i][:, 128 * m:128 * (m + 1)],
                                 h1[:, i, :], start=(i == 0), stop=False)
            nc.tensor.matmul(ph[:], fb2row[:, 128 * m:128 * (m + 1)], ones2[:],
                             start=False, stop=True)
            nc.scalar.activation(h2[:, m, :], ph[:], AF.Copy)
        bn_head(h2, 2, fg2_d, fbb2_d, "h2")
        lg = wk.tile([2, CLS], F32, tag="G", bufs=3)
        for ch, (c0, cw) in enumerate([(0, 512), (512, CLS - 512)]):
            pl = ps.tile([2, 512], F32, name=f"pl{ch}", tag="ps")
            for i in range(2):
                nc.tensor.matmul(pl[:, 0:cw], h2[:, i, :], fw3_sb[i][:, c0:c0 + cw],
                                 start=(i == 0), stop=False)
            nc.tensor.matmul(pl[:, 0:cw], ones2[:], fb3row[:, c0:c0 + cw],
                             start=False, stop=True)
            nc.scalar.activation(lg[:, c0:c0 + cw], pl[:, 0:cw], AF.Copy)
        mxl = wk.tile([2, 4], F32, tag="mxl")
        nc.vector.tensor_reduce(mxl[:, 0:1], lg[:], AX.X, ALU.max)
        nc.vector.tensor_scalar(mxl[:, 1:2], mxl[:, 0:1], -1.0, None, ALU.mult)
        ex = wk.tile([2, CLS], F32, tag="G", bufs=3)
        nc.scalar.activation(ex[:], lg[:], AF.Exp, bias=mxl[:, 1:2],
                             accum_out=mxl[:, 2:3])
        nc.scalar.activation(mxl[:, 3:4], mxl[:, 2:3], AF.Ln)
        nc.vector.tensor_add(mxl[:, 3:4], mxl[:, 3:4], mxl[:, 0:1])
        nc.vector.tensor_scalar(mxl[:, 3:4], mxl[:, 3:4], -1.0, None, ALU.mult)
        outt = wk.tile([2, CLS], F32, tag="G", bufs=3)
        nc.scalar.activation(outt[:], lg[:], AF.Identity, bias=mxl[:, 3:4])
        nc.sync.dma_start(out_d[:], outt[:])

    nc.compile()
    return nc


_NC_CACHE = None


def build_in_maps(inputs):
    pts = np.asarray(inputs["pts"], dtype=np.float32)

    def T(x):
        return np.ascontiguousarray(np.asarray(x, dtype=np.float32).T)

    base = {}
    for li in range(4):
        C = LAYERS[li][0]
        w = np.asarray(inputs[f"w{li + 1}"], dtype=np.float32)
        base[f"wn{li}"] = np.ascontiguousarray(w[:, :C].T)
        base[f"wd{li}"] = np.ascontiguousarray((w[:, C:] - w[:, :C]).T)
        base[f"g{li}"] = np.asarray(inputs[f"g{li + 1}"], dtype=np.float32)
        base[f"b{li}"] = np.asarray(inputs[f"b{li + 1}"], dtype=np.float32)
    base["w5T"] = T(inputs["w5"])
    base["g5"] = np.asarray(inputs["g5"], dtype=np.float32)
    base["b5"] = np.asarray(inputs["b5"], dtype=np.float32)
    base["fw1T"] = np.vstack([T(inputs["fw1"]),
                              np.asarray(inputs["fb1"], np.float32)[None, :]])
    base["fg1"] = np.asarray(inputs["fg1"], np.float32)
    base["fbb1"] = np.asarray(inputs["fbb1"], np.float32)
    base["fw2T"] = np.vstack([T(inputs["fw2"]),
                              np.asarray(inputs["fb2"], np.float32)[None, :]])
    base["fg2"] = np.asarray(inputs["fg2"], np.float32)
    base["fbb2"] = np.asarray(inputs["fbb2"], np.float32)
    base["fw3T"] = np.vstack([T(inputs["fw3"]),
                              np.asarray(inputs["fb3"], np.float32)[None, :]])

    in_maps = []
    for c in range(N_CORES):
        m = dict(base)
        shard = pts[c * BL:(c + 1) * BL]
        m["ptsT"] = np.ascontiguousarray(shard.transpose(0, 2, 1))
        in_maps.append(m)
    return in_maps


def kernel(**inputs):
    global _NC_CACHE
    if _NC_CACHE is None:
        _NC_CACHE = build()
    nc = _NC_CACHE
    in_maps = build_in_maps(inputs)
    res = bass_utils.run_bass_kernel_spmd(nc, in_maps, core_ids=list(range(N_CORES)))
    out = np.concatenate([res.results[c]["out"] for c in range(N_CORES)], axis=0)
    return out.astype(np.float32)
